# revision 1
# baseline (speedup 1.0000x reference)
"""Trainium2 Bass kernel for nn_Colar_static (retrieval_knn).

Strategy: data-parallel over batch B across 8 cores; prototype
projections (Ek/Ev) replicated per core. Everything on device runs in
a "transposed" orientation with the batch on the free dimension and
channels / prototype-columns on partitions, so that:
  - k-row norms / softmax sums are PE ones-matmuls (partition reduce),
  - Ek column norms and per-(k,n) exp scaling are per-partition scalars,
  - no on-device transposes are needed anywhere.
SBUF singles are created in reverse order of death (LIFO pool stack).
"""

import os
import sys

for _p in ("/opt/trn_rl_repo", "/opt/pypackages"):
    if _p not in sys.path:
        sys.path.append(_p)

import numpy as np
import ml_dtypes

import concourse.bass as bass
import concourse.mybir as mybir
import concourse.tile as tile
from concourse import bacc
from concourse import bass_utils

B, T, CH, C, N, K = 4096, 8, 2048, 1024, 512, 5
NCORES = 8
BL = B // NCORES            # 512 batch rows per core
KN = K * N                  # 2560 prototype columns
P = 128
NT_I = CH // P              # 16 contraction tiles (input channels)
NT_C = C // P               # 8 tiles over C
NT_KN = KN // P             # 20 tiles over K*N
NT_KV = 2 * C // P          # 16 tiles over [k|v] output channels
TPK = NT_KN // K            # 4 kn-tiles per prototype
EPS = 1e-8

F32 = mybir.dt.float32
BF16 = mybir.dt.bfloat16
AF = mybir.ActivationFunctionType
MUL = mybir.AluOpType.mult
ADD = mybir.AluOpType.add

_CACHE = {}


def _build_nc():
    PH = int(os.environ.get("KPHASES", "9"))
    KA1 = int(os.environ.get("KA1", "9"))
    nc = bacc.Bacc(None, target_bir_lowering=False, debug=False)

    xT = nc.dram_tensor("xT", [CH, BL], BF16, kind="ExternalInput")
    wkvT = nc.dram_tensor("wkvT", [CH, 2 * C], BF16, kind="ExternalInput")
    wekT = nc.dram_tensor("wekT", [CH, C], BF16, kind="ExternalInput")
    wevT = nc.dram_tensor("wevT", [CH, C], BF16, kind="ExternalInput")
    statf = nc.dram_tensor("statf", [CH, KN], BF16, kind="ExternalInput")
    bek = nc.dram_tensor("bek", [P, NT_C], F32, kind="ExternalInput")
    bkv = nc.dram_tensor("bkv", [P, NT_KV], F32, kind="ExternalInput")
    bev = nc.dram_tensor("bev", [P, C], F32, kind="ExternalInput")
    wwb = nc.dram_tensor("wwb", [P, C], BF16, kind="ExternalInput")
    wout = nc.dram_tensor("wout", [P, NT_KV * K], BF16, kind="ExternalInput")
    bws = nc.dram_tensor("bws", [1, 1], F32, kind="ExternalInput")
    boutt = nc.dram_tensor("boutt", [K, 1], F32, kind="ExternalInput")
    outT = nc.dram_tensor("outT", [K, BL], F32, kind="ExternalOutput")
    # DRAM scratch as ExternalOutputs: Internal DRAM tiles are compiled with
    # physical addresses (--mem-mode=physical) and wedge the device when the
    # NEFF is loaded via the PJRT/axon path; External allocations relocate.
    evspill = nc.dram_tensor("evs", [NT_KN, P, C], BF16, kind="ExternalOutput")
    invbounce = nc.dram_tensor("invb", [1, KN], F32, kind="ExternalOutput")

    tc_cm = tile.TileContext(nc)
    tc = tc_cm.__enter__()
    if True:
        if True:
            # ---- engine warmups: first use of an ACT table costs ~64us
            # (three table loads ~192us total); issue tiny activations up
            # front so the loads overlap the initial weight DMAs.
            warm, f_warm = tc.tile([1, 16], F32, name="warm")
            nc.vector.memset(warm[:], 1.0)
            for wf_i, wfunc in enumerate((AF.Identity, AF.Square, AF.Relu,
                                          AF.Exp, AF.Sqrt, AF.Sigmoid)):
                wo_t, f_wo_t = tc.tile([1, 16], F32, name=f"warmo{wf_i}")
                nc.scalar.activation(wo_t[:], warm[:], wfunc)
                f_wo_t()
            f_warm()

            # ---- persistents (die at the very end), bottom of pool stack
            epsb, _f0 = tc.tile([1, 1], F32, name="epsb")
            nc.vector.memset(epsb[:], EPS * EPS)
            ones_col, _f1 = tc.tile([P, 1], BF16, name="ones_col")
            nc.any.memset(ones_col[:], 1.0)
            ones_row, _f2 = tc.tile([1, P], F32, name="ones_row")
            nc.any.memset(ones_row[:], 1.0)
            bek_sb, _f3 = tc.tile([P, NT_C], F32, name="bek_sb")
            nc.sync.dma_start(bek_sb[:], bek[:])
            bkv_sb, _f4 = tc.tile([P, NT_KV], F32, name="bkv_sb")
            nc.sync.dma_start(bkv_sb[:], bkv[:])
            bw_sb, _f5 = tc.tile([1, 1], F32, name="bw_sb")
            nc.sync.dma_start(bw_sb[:], bws[:])
            bout_sb, _f6 = tc.tile([K, 1], F32, name="bout_sb")
            nc.sync.dma_start(bout_sb[:], boutt[:])
            wo_sb, _f7 = tc.tile([P, NT_KV * K], BF16, name="wo_sb")
            nc.sync.dma_start(wo_sb[:], wout[:])

            # dies OUT-end
            vr_all, f_vr = tc.tile([P, NT_C, BL], BF16, name="vr_all")
            fr_all, f_fr = tc.tile([P, NT_C, BL], BF16, name="fr_all")
            # dies GATE-end (written in A2)
            wevA, f_wevA = tc.tile([P, NT_KN], F32, name="wevA")
            wevB, f_wevB = tc.tile([P, NT_KN], F32, name="wevB")
            # dies FE-end (written in fused sim/gate phase)
            wf_all, f_wf = tc.tile([P, NT_KN, BL], BF16, name="wf_all")
            # dies SIM-end (written in A1)
            ek_all, f_ek = tc.tile([P, NT_C, KN], BF16, name="ek_all")
            # dies A1-end (used by A2 as lhsT and A1 as rhs)
            st_all, f_st = tc.tile([P, NT_I, KN], BF16, name="st_all")
            for i in range(NT_I):
                nc.sync.dma_start(st_all[:, i, :], statf[i * P:(i + 1) * P, :])

            # ============ Phase A2: EvT[kn, c] -> DRAM spill, wEv =========
            bev_sb, f_bev = tc.tile([P, C], F32, name="bev_sb")
            nc.sync.dma_start(bev_sb[:], bev[:])
            ww_sb, f_ww = tc.tile([P, C], BF16, name="ww_sb")
            nc.sync.dma_start(ww_sb[:], wwb[:])

            with tc.tile_pool(name="a2w", bufs=3) as a2w, \
                 tc.tile_pool(name="wvp", bufs=1) as wvp, \
                 tc.tile_pool(name="pa2", bufs=3, space="PSUM") as pa2:
                for cc in range(2):
                    wv_h = wvp.tile([P, NT_I, 512], BF16, tag="wvh")
                    for i in range(NT_I):
                        nc.sync.dma_start(
                            wv_h[:, i, :],
                            wevT[i * P:(i + 1) * P, cc * 512:(cc + 1) * 512])
                    wev_half = wevA if cc == 0 else wevB
                    for kt in range(NT_KN):
                        ps = pa2.tile([P, 512], F32, tag="a2ps")
                        for i in range(NT_I):
                            nc.tensor.matmul(
                                ps[:],
                                st_all[:, i, kt * P:(kt + 1) * P],
                                wv_h[:, i, :],
                                start=(i == 0), stop=(i == NT_I - 1))
                        evt_bf = a2w.tile([P, 512], BF16, tag="evtbf")
                        nc.vector.tensor_add(
                            evt_bf[:], ps[:],
                            bev_sb[:, cc * 512:(cc + 1) * 512])
                        scr = a2w.tile([P, 512], BF16, tag="a2scr")
                        nc.vector.tensor_mul(
                            scr[:], evt_bf[:],
                            ww_sb[:, cc * 512:(cc + 1) * 512])
                        nc.vector.tensor_reduce(
                            wev_half[:, kt:kt + 1], scr[:],
                            axis=mybir.AxisListType.X, op=ADD)
                        nc.sync.dma_start(
                            evspill[kt, :, cc * 512:(cc + 1) * 512],
                            evt_bf[:])
            f_ww()
            f_bev()
            if PH < 2:
                f_st(); f_ek(); f_wf(); f_wevB(); f_wevA(); f_fr(); f_vr()
                _f7(); _f6(); _f5(); _f4(); _f3(); _f2(); _f1()
                tc_cm.__exit__(None, None, None)
                nc.compile()
                return nc

            # ============ Phase A1: Ek[c, kn] + invnormE -> DRAM bounce ===
            NCH = KN // 512
            with tc.tile_pool(name="a1w", bufs=3) as a1w, \
                 tc.tile_pool(name="wep", bufs=1) as wep, \
                 tc.tile_pool(name="pa1", bufs=2, space="PSUM") as pa1, \
                 tc.tile_pool(name="pss", bufs=1, space="PSUM") as pss:
                ss = [pss.tile([1, 512], F32, name=f"ss{j}") for j in range(NCH)]
                if KA1 < 1:
                    for j in range(NCH):
                        nc.vector.memset(ss[j], 0.0)
                for wh in range(2):
                    we_h = wep.tile([P, NT_I, 512], BF16, tag="weh")
                    for i in range(NT_I):
                        nc.sync.dma_start(
                            we_h[:, i, :],
                            wekT[i * P:(i + 1) * P, wh * 512:(wh + 1) * 512])
                    for ml in range(NT_C // 2):
                        m = wh * (NT_C // 2) + ml
                        for nch in range(NCH):
                            ps = pa1.tile([P, 512], F32, tag="a1ps")
                            for i in range(NT_I):
                                nc.tensor.matmul(
                                    ps[:],
                                    we_h[:, i, ml * P:(ml + 1) * P],
                                    st_all[:, i, nch * 512:(nch + 1) * 512],
                                    start=(i == 0), stop=(i == NT_I - 1))
                            nc.scalar.activation(
                                ek_all[:, m, nch * 512:(nch + 1) * 512],
                                ps[:], AF.Identity, bias=bek_sb[:, m:m + 1])
                            if KA1 >= 1:
                                sq = a1w.tile([P, 512], BF16, tag="a1sq")
                                nc.scalar.activation(
                                    sq[:], ps[:], AF.Square,
                                    bias=bek_sb[:, m:m + 1])
                                nc.tensor.matmul(
                                    ss[nch], ones_col[:], sq[:],
                                    start=(m == 0), stop=(m == NT_C - 1))
                for j in range(NCH):
                    if KA1 < 1:
                        break
                    if KA1 < 2:
                        tmp = a1w.tile([1, 512], F32, tag="nrow")
                        nc.scalar.copy(tmp[:], ss[j])
                        if int(os.environ.get("KINVDMA", "1")):
                            nc.sync.dma_start(
                                invbounce[0:1, j * 512:(j + 1) * 512],
                                tmp[0:1, :])
                        continue
                    nrow = a1w.tile([1, 512], F32, tag="nrow")
                    nc.scalar.activation(nrow[:], ss[j], AF.Sqrt,
                                         bias=epsb[:])
                    invrow = a1w.tile([1, 512], F32, tag="invrow")
                    nc.vector.reciprocal(invrow[:], nrow[:])
                    nc.sync.dma_start(invbounce[0:1, j * 512:(j + 1) * 512],
                                      invrow[0:1, :])
            f_st()
            if PH < 3:
                f_ek(); f_wf(); f_wevB(); f_wevA(); f_fr(); f_vr()
                _f7(); _f6(); _f5(); _f4(); _f3(); _f2(); _f1()
                tc_cm.__exit__(None, None, None)
                nc.compile()
                return nc

            # ============ Phase KV: normalized kT, relu(vT) ==============
            # creation order = reverse death: kn_all & inv_col die SIM-end,
            # kT/sqk die after the kn mult, xp dies when kv matmuls finish.
            kn_all, f_kn = tc.tile([P, NT_C, BL], BF16, name="kn_all")
            inv_col, f_inv = tc.tile([P, NT_KN], F32, name="inv_col")
            nc.sync.dma_start(
                inv_col[:], invbounce[0, :].rearrange("(j p) -> p j", p=P))
            kT_all, f_kT = tc.tile([P, NT_C, BL], F32, name="kT_all")
            sqk_all, f_sqk = tc.tile([P, NT_C, BL], BF16, name="sqk_all")
            xp_all, f_xp = tc.tile([P, NT_I, BL], BF16, name="xp_all")
            for i in range(NT_I):
                nc.sync.dma_start(xp_all[:, i, :], xT[i * P:(i + 1) * P, :])

            with tc.tile_pool(name="wkvp", bufs=2) as wkvp, \
                 tc.tile_pool(name="pkv", bufs=2, space="PSUM") as pkv:
                for mg in range(4):
                    kv_ps = [pkv.tile([P, BL], F32, tag=f"kvps{q}",
                                      name=f"kvps{mg}_{q}")
                             for q in range(4)]
                    for i in range(NT_I):
                        wp = wkvp.tile([P, 512], BF16, tag="wp")
                        nc.sync.dma_start(
                            wp[:], wkvT[i * P:(i + 1) * P,
                                        mg * 512:(mg + 1) * 512])
                        for q in range(4):
                            nc.tensor.matmul(
                                kv_ps[q], wp[:, q * P:(q + 1) * P],
                                xp_all[:, i, :],
                                start=(i == 0), stop=(i == NT_I - 1))
                    for q in range(4):
                        m = mg * 4 + q
                        if m < NT_C:
                            nc.scalar.activation(
                                kT_all[:, m, :], kv_ps[q], AF.Identity,
                                bias=bkv_sb[:, m:m + 1])
                            nc.scalar.activation(
                                sqk_all[:, m, :], kv_ps[q], AF.Square,
                                bias=bkv_sb[:, m:m + 1])
                        else:
                            nc.scalar.activation(
                                vr_all[:, m - NT_C, :], kv_ps[q], AF.Relu,
                                bias=bkv_sb[:, m:m + 1])
            f_xp()

            with tc.tile_pool(name="kvw", bufs=2) as kvw, \
                 tc.tile_pool(name="pssk", bufs=1, space="PSUM") as pssk, \
                 tc.tile_pool(name="pbc", bufs=1, space="PSUM") as pbc:
                ssk = pssk.tile([1, BL], F32)
                for m in range(NT_C):
                    nc.tensor.matmul(ssk[:], ones_col[:], sqk_all[:, m, :],
                                     start=(m == 0), stop=(m == NT_C - 1))
                nk = kvw.tile([1, BL], F32, tag="nk")
                nc.scalar.activation(nk[:], ssk[:], AF.Sqrt, bias=epsb[:])
                invk = kvw.tile([1, BL], F32, tag="invk")
                nc.vector.reciprocal(invk[:], nk[:])
                bc = pbc.tile([P, BL], F32)
                nc.tensor.matmul(bc[:], ones_row[:], invk[:])
                for m in range(NT_C):
                    nc.vector.tensor_mul(kn_all[:, m, :], kT_all[:, m, :],
                                         bc[:])
            f_sqk()
            f_kT()
            if PH < 4:
                nc.sync.dma_start(evspill[0, :, 0:BL], kn_all[:, 0, :])
                f_xp2_unused = None
                f_inv(); f_kn(); f_ek(); f_wf(); f_wevB(); f_wevA(); f_fr(); f_vr()
                _f7(); _f6(); _f5(); _f4(); _f3(); _f2(); _f1()
                tc_cm.__exit__(None, None, None)
                nc.compile()
                return nc

            # ============ Fused SIM + GATE + WF ==========================
            with tc.tile_pool(name="gw", bufs=2) as gw, \
                 tc.tile_pool(name="esw", bufs=8) as esw, \
                 tc.tile_pool(name="psim", bufs=3, space="PSUM") as psim, \
                 tc.tile_pool(name="pg", bufs=1, space="PSUM") as pg, \
                 tc.tile_pool(name="pbc2", bufs=2, space="PSUM") as pbc2:
                wev_sum = gw.tile([P, NT_KN], F32, tag="wevsum")
                nc.vector.tensor_add(wev_sum[:], wevA[:], wevB[:])
                wev_bf = gw.tile([P, NT_KN], BF16, tag="wevbf")
                nc.vector.tensor_copy(wev_bf[:], wev_sum[:])
                for k in range(K):
                    gse = pg.tile([1, BL], F32, tag="gse")
                    gtg = pg.tile([1, BL], F32, tag="gtg")
                    es_list = []
                    for j in range(TPK):
                        kt = k * TPK + j
                        ps = psim.tile([P, BL], F32, tag="simps")
                        for m in range(NT_C):
                            nc.tensor.matmul(
                                ps[:], ek_all[:, m, kt * P:(kt + 1) * P],
                                kn_all[:, m, :],
                                start=(m == 0), stop=(m == NT_C - 1))
                        es = esw.tile([P, BL], BF16, tag="esw")
                        nc.scalar.activation(es[:], ps[:], AF.Exp,
                                             scale=inv_col[:, kt:kt + 1])
                        es_list.append(es)
                        nc.tensor.matmul(gse[:], ones_col[:], es[:],
                                         start=(j == 0), stop=(j == TPK - 1))
                        nc.tensor.matmul(gtg[:], wev_bf[:, kt:kt + 1], es[:],
                                         start=(j == 0), stop=(j == TPK - 1))
                    rs = gw.tile([1, BL], F32, tag="rs")
                    nc.vector.reciprocal(rs[:], gse[:])
                    tg = gw.tile([1, BL], F32, tag="tg")
                    nc.vector.tensor_mul(tg[:], gtg[:], rs[:])
                    fwk = gw.tile([1, BL], F32, tag="fwk")
                    nc.scalar.activation(fwk[:], tg[:], AF.Sigmoid,
                                         bias=bw_sb[0:1, 0:1])
                    sk = gw.tile([1, BL], F32, tag="sk")
                    nc.vector.tensor_mul(sk[:], fwk[:], rs[:])
                    bcs = pbc2.tile([P, BL], F32, tag="bcs")
                    nc.tensor.matmul(bcs[:], ones_row[:], sk[:])
                    bcs_sb = gw.tile([P, BL], BF16, tag="bcssb")
                    nc.scalar.copy(bcs_sb[:], bcs[:])
                    for j in range(TPK):
                        kt = k * TPK + j
                        nc.vector.tensor_mul(wf_all[:, kt, :], es_list[j],
                                             bcs_sb[:])
            f_inv()
            f_kn()
            f_ek()
            if PH < 5:
                nc.sync.dma_start(evspill[0, :, 0:BL], wf_all[:, 0, :])
                f_wf(); f_wevB(); f_wevA(); f_fr(); f_vr()
                _f7(); _f6(); _f5(); _f4(); _f3(); _f2(); _f1()
                tc_cm.__exit__(None, None, None)
                nc.compile()
                return nc

            # ============ Phase FE ========================================
            evt_all, f_evt = tc.tile([P, NT_KN, C], BF16, name="evt_all")
            for kt in range(NT_KN):
                nc.sync.dma_start(evt_all[:, kt, :], evspill[kt])
            with tc.tile_pool(name="pfe", bufs=3, space="PSUM") as pfe:
                for mc in range(NT_C):
                    ps = pfe.tile([P, BL], F32, tag="feps")
                    for kt in range(NT_KN):
                        nc.tensor.matmul(
                            ps[:], evt_all[:, kt, mc * P:(mc + 1) * P],
                            wf_all[:, kt, :],
                            start=(kt == 0), stop=(kt == NT_KN - 1))
                    nc.scalar.activation(fr_all[:, mc, :], ps[:], AF.Relu)
            f_evt()
            f_wf()
            f_wevB()
            f_wevA()

            # ============ Phase OUT =======================================
            with tc.tile_pool(name="ow", bufs=1) as ow, \
                 tc.tile_pool(name="pout", bufs=1, space="PSUM") as pout:
                po = pout.tile([K, BL], F32)
                for j in range(NT_KV):
                    rhs = vr_all[:, j, :] if j < NT_C else \
                        fr_all[:, j - NT_C, :]
                    nc.tensor.matmul(po[:], wo_sb[:, j * K:(j + 1) * K], rhs,
                                     start=(j == 0), stop=(j == NT_KV - 1))
                osb = ow.tile([K, BL], F32)
                nc.scalar.activation(osb[:], po[:], AF.Identity,
                                     bias=bout_sb[:])
                nc.sync.dma_start(outT[:], osb[:])
            f_fr()
            f_vr()
            _f7()
            _f6()
            _f5()
            _f4()
            _f3()
            _f2()
            _f1()
            _f0()

    tc_cm.__exit__(None, None, None)
    nc.compile()
    return nc


def _host_prep(inputs):
    bf = ml_dtypes.bfloat16
    x_last = np.asarray(inputs["x"])[:, -1, :]  # [B, CH] f32
    shared = {
        "wkvT": np.ascontiguousarray(
            np.concatenate([inputs["Wk"], inputs["Wv"]], axis=0).T
        ).astype(bf),
        "wekT": np.ascontiguousarray(np.asarray(inputs["WEk"]).T).astype(bf),
        "wevT": np.ascontiguousarray(np.asarray(inputs["WEv"]).T).astype(bf),
        "statf": np.ascontiguousarray(
            np.asarray(inputs["static"]).transpose(1, 0, 2).reshape(CH, KN)
        ).astype(bf),
        "bek": np.ascontiguousarray(
            np.asarray(inputs["bEk"]).reshape(NT_C, P).T),
        "bkv": np.ascontiguousarray(
            np.concatenate([inputs["bk"], inputs["bv"]]).reshape(NT_KV, P).T),
        "bev": np.ascontiguousarray(
            np.broadcast_to(np.asarray(inputs["bEv"]), (P, C))),
        "wwb": np.ascontiguousarray(
            np.broadcast_to(np.asarray(inputs["Ww"])[0], (P, C))).astype(bf),
        "wout": np.ascontiguousarray(
            np.asarray(inputs["Wout"]).T.reshape(NT_KV, P, K)
            .transpose(1, 0, 2).reshape(P, NT_KV * K)).astype(bf),
        "bws": np.asarray(inputs["bw"], dtype=np.float32).reshape(1, 1),
        "boutt": np.asarray(inputs["bout"], dtype=np.float32).reshape(K, 1),
    }
    in_maps = []
    for r in range(NCORES):
        m = dict(shared)
        m["xT"] = np.ascontiguousarray(
            x_last[r * BL:(r + 1) * BL].T).astype(bf)
        in_maps.append(m)
    return in_maps


def kernel(**inputs):
    if "nc" not in _CACHE:
        _CACHE["nc"] = _build_nc()
    nc = _CACHE["nc"]
    in_maps = _host_prep(inputs)
    res = bass_utils.run_bass_kernel_spmd(
        nc, in_maps, core_ids=list(range(NCORES)), trace=False)
    out = np.concatenate(
        [res.results[r]["outT"].T for r in range(NCORES)], axis=0)
    return np.ascontiguousarray(out[:, :, None], dtype=np.float32)



# revision 8
# speedup vs baseline: 1.4704x; 1.4704x over previous
"""Trainium2 Bass kernel for nn_Colar_static (retrieval_knn).

Strategy: data-parallel over batch B across 8 cores PLUS tensor-parallel
split of the Ek/Ev prototype projections over the C=1024 channel dim
(each core computes a [128, K*N] slab = 1/8 of the work the baseline
replicated). Slabs are exchanged with on-chip collectives:
  - AllReduce  [2, KN] f32   : Ek column sum-of-squares + wEv gate row
  - AllGather  [128, KN] bf16: Ek slab (c-tile per rank)
  - AllGather  [128, KN] bf16: Ev^T slab (kn on partitions, c-slice free)
Collectives overlap the batch-local k/v projection phase. Everything
runs "transposed" with batch on the free dimension, as in the baseline.
SBUF singles are created in reverse order of death (LIFO pool stack).
"""

import sys

for _p in ("/opt/trn_rl_repo", "/opt/pypackages"):
    if _p not in sys.path:
        sys.path.append(_p)

import numpy as np
import ml_dtypes

import concourse.bass as bass
import concourse.mybir as mybir
import concourse.tile as tile
from concourse import bacc
from concourse import bass_utils

B, T, CH, C, N, K = 4096, 8, 2048, 1024, 512, 5
NCORES = 8
BL = B // NCORES            # 512 batch rows per core
KN = K * N                  # 2560 prototype columns
P = 128
NT_I = CH // P              # 16 contraction tiles (input channels)
NT_C = C // P               # 8 tiles over C
NT_KN = KN // P             # 20 tiles over K*N
NT_KV = 2 * C // P          # 16 tiles over [k|v] output channels
TPK = NT_KN // K            # 4 kn-tiles per prototype
NCH = KN // 512             # 5 column chunks for the slab projections
EPS = 1e-8

F32 = mybir.dt.float32
BF16 = mybir.dt.bfloat16
AF = mybir.ActivationFunctionType
MUL = mybir.AluOpType.mult
ADD = mybir.AluOpType.add

_CACHE = {}


def _build_nc():
    nc = bacc.Bacc(None, target_bir_lowering=False, debug=False)

    xT = nc.dram_tensor("xT", [CH, BL], BF16, kind="ExternalInput")
    wkvT = nc.dram_tensor("wkvT", [CH, 2 * C], BF16, kind="ExternalInput")
    wekc = nc.dram_tensor("wekc", [CH, P], BF16, kind="ExternalInput")
    wevc = nc.dram_tensor("wevc", [CH, P], BF16, kind="ExternalInput")
    statf = nc.dram_tensor("statf", [CH, KN], BF16, kind="ExternalInput")
    bekc = nc.dram_tensor("bekc", [P, 1], F32, kind="ExternalInput")
    bevc = nc.dram_tensor("bevc", [P, 1], F32, kind="ExternalInput")
    wwc = nc.dram_tensor("wwc", [P, 1], BF16, kind="ExternalInput")
    bkv = nc.dram_tensor("bkv", [P, NT_KV], F32, kind="ExternalInput")
    ident = nc.dram_tensor("ident", [P, P], BF16, kind="ExternalInput")
    wout = nc.dram_tensor("wout", [P, NT_KV * K], BF16, kind="ExternalInput")
    bws = nc.dram_tensor("bws", [1, 1], F32, kind="ExternalInput")
    boutt = nc.dram_tensor("boutt", [K, 1], F32, kind="ExternalInput")
    outT = nc.dram_tensor("outT", [K, BL], F32, kind="ExternalOutput")

    # collective bounce buffers (inputs must be Internal-Local; gather
    # outputs Shared so ranks deposit slices into one HBM buffer)
    ccr_in = nc.dram_tensor("ccr_in", [2, KN], F32)
    ccr_out = nc.dram_tensor("ccr_out", [2, KN], F32)
    ccek_in = nc.dram_tensor("ccek_in", [P, KN], BF16)
    ccek_out = nc.dram_tensor("ccek_out", [NCORES, P, KN], BF16,
                              addr_space="Shared")
    ccevt_in = nc.dram_tensor("ccevt_in", [P, KN], BF16)
    ccevt_out = nc.dram_tensor("ccevt_out", [NCORES, P, KN], BF16,
                               addr_space="Shared")
    GROUPS = [list(range(NCORES))]

    tc_cm = tile.TileContext(nc)
    tc = tc_cm.__enter__()

    # ---- engine warmups: first use of an ACT table costs ~64us; issue
    # tiny activations up front so table loads overlap the initial DMAs.
    warm, f_warm = tc.tile([1, 16], F32, name="warm")
    nc.vector.memset(warm[:], 1.0)
    for wf_i, wfunc in enumerate((AF.Identity, AF.Square, AF.Relu,
                                  AF.Exp, AF.Sqrt, AF.Sigmoid)):
        wo_t, f_wo_t = tc.tile([1, 16], F32, name=f"warmo{wf_i}")
        nc.scalar.activation(wo_t[:], warm[:], wfunc)
        f_wo_t()
    f_warm()

    # ---- persistents (die at the very end), bottom of pool stack
    epsb, _f0 = tc.tile([1, 1], F32, name="epsb")
    nc.vector.memset(epsb[:], EPS * EPS)
    epsb_p, _f0b = tc.tile([P, 1], F32, name="epsb_p")
    nc.vector.memset(epsb_p[:], EPS * EPS)
    ones_col, _f1 = tc.tile([P, 1], BF16, name="ones_col")
    nc.any.memset(ones_col[:], 1.0)
    ones_row, _f2 = tc.tile([1, P], F32, name="ones_row")
    nc.any.memset(ones_row[:], 1.0)
    bkv_sb, _f3 = tc.tile([P, NT_KV], F32, name="bkv_sb")
    nc.sync.dma_start(bkv_sb[:], bkv[:])
    bw_sb, _f4 = tc.tile([1, 1], F32, name="bw_sb")
    nc.sync.dma_start(bw_sb[:], bws[:])
    bout_sb, _f5 = tc.tile([K, 1], F32, name="bout_sb")
    nc.sync.dma_start(bout_sb[:], boutt[:])
    wo_sb, _f6 = tc.tile([P, NT_KV * K], BF16, name="wo_sb")
    nc.sync.dma_start(wo_sb[:], wout[:])

    # dies OUT-end
    vr_all, f_vr = tc.tile([P, NT_C, BL], BF16, name="vr_all")
    fr_all, f_fr = tc.tile([P, NT_C, BL], BF16, name="fr_all")
    # dies FE-end
    wf_all, f_wf = tc.tile([P, NT_KN, BL], BF16, name="wf_all")
    # die SIM-end (written after collectives)
    lhs2, f_lhs2 = tc.tile([P, NT_KN, 2], BF16, name="lhs2")
    inv_col, f_inv = tc.tile([P, NT_KN], F32, name="inv_col")
    kn_all, f_kn = tc.tile([P, NT_C, BL], BF16, name="kn_all")

    # ============ Phase P: Ek / Ev^T slabs (this core's 128 c's) =====
    wek_sb, f_wek = tc.tile([P, NT_I, P], BF16, name="wek_sb")
    wev_sb, f_wev = tc.tile([P, NT_I, P], BF16, name="wev_sb")
    for i in range(NT_I):
        nc.sync.dma_start(wek_sb[:, i, :], wekc[i * P:(i + 1) * P, :])
        nc.sync.dma_start(wev_sb[:, i, :], wevc[i * P:(i + 1) * P, :])
    bekc_sb, f_bek = tc.tile([P, 1], F32, name="bekc_sb")
    nc.sync.dma_start(bekc_sb[:], bekc[:])
    bevc_sb, f_bev = tc.tile([P, 1], F32, name="bevc_sb")
    nc.sync.dma_start(bevc_sb[:], bevc[:])
    wwc_sb, f_ww = tc.tile([P, 1], BF16, name="wwc_sb")
    nc.sync.dma_start(wwc_sb[:], wwc[:])
    id_sb, f_id = tc.tile([P, P], BF16, name="id_sb")
    nc.sync.dma_start(id_sb[:], ident[:])
    ek_slab, f_eks = tc.tile([P, KN], BF16, name="ek_slab")
    evt_slab, f_evs = tc.tile([P, KN], BF16, name="evt_slab")

    with tc.tile_pool(name="stp", bufs=2) as stp, \
         tc.tile_pool(name="pw", bufs=3) as pw, \
         tc.tile_pool(name="rowp", bufs=2) as rowp, \
         tc.tile_pool(name="ppk", bufs=2, space="PSUM") as ppk, \
         tc.tile_pool(name="prow", bufs=1, space="PSUM") as prow, \
         tc.tile_pool(name="ptp", bufs=2, space="PSUM") as ptp:
        for ch in range(NCH):
            cs = slice(ch * 512, (ch + 1) * 512)
            stch = stp.tile([P, NT_I, 512], BF16, tag="st")
            for i in range(NT_I):
                nc.sync.dma_start(stch[:, i, :],
                                  statf[i * P:(i + 1) * P, cs])
            # Ek chunk
            ek_ps = ppk.tile([P, 512], F32, tag="ek")
            for i in range(NT_I):
                nc.tensor.matmul(ek_ps[:], wek_sb[:, i, :], stch[:, i, :],
                                 start=(i == 0), stop=(i == NT_I - 1))
            nc.scalar.activation(ek_slab[:, cs], ek_ps[:], AF.Identity,
                                 bias=bekc_sb[:])
            sqt = pw.tile([P, 512], BF16, tag="sq")
            nc.scalar.activation(sqt[:], ek_ps[:], AF.Square,
                                 bias=bekc_sb[:])
            sq_ps = prow.tile([1, 512], F32, tag="row")
            nc.tensor.matmul(sq_ps[:], ones_col[:], sqt[:])
            sq_row = rowp.tile([1, 512], F32, tag="sqr")
            nc.vector.tensor_copy(sq_row[:], sq_ps[:])
            nc.sync.dma_start(ccr_in[0:1, cs], sq_row[:])
            # Ev chunk
            ev_ps = ppk.tile([P, 512], F32, tag="ev")
            for i in range(NT_I):
                nc.tensor.matmul(ev_ps[:], wev_sb[:, i, :], stch[:, i, :],
                                 start=(i == 0), stop=(i == NT_I - 1))
            evbf = pw.tile([P, 512], BF16, tag="ev")
            nc.scalar.activation(evbf[:], ev_ps[:], AF.Identity,
                                 bias=bevc_sb[:])
            wev_ps = prow.tile([1, 512], F32, tag="row")
            nc.tensor.matmul(wev_ps[:], wwc_sb[:], evbf[:])
            wev_row = rowp.tile([1, 512], F32, tag="wvr")
            nc.vector.tensor_copy(wev_row[:], wev_ps[:])
            nc.sync.dma_start(ccr_in[1:2, cs], wev_row[:])
            # Ev^T chunk (4 PE transposes via identity)
            tp_ps = ptp.tile([P, 512], BF16, tag="tp")
            for q in range(4):
                nc.tensor.transpose(tp_ps[:, q * P:(q + 1) * P],
                                    evbf[:, q * P:(q + 1) * P], id_sb[:])
            nc.scalar.copy(evt_slab[:, cs], tp_ps[:])
        nc.sync.dma_start(ccek_in[:], ek_slab[:])
        nc.sync.dma_start(ccevt_in[:], evt_slab[:])
    f_evs()
    f_eks()
    f_id()
    f_ww()
    f_bev()
    f_bek()
    f_wev()
    f_wek()

    # ============ Collectives (overlap the KV phase) ==================
    nc.gpsimd.collective_compute(
        "AllReduce", mybir.AluOpType.add, replica_groups=GROUPS,
        ins=[ccr_in[:].opt()], outs=[ccr_out[:].opt()])
    nc.gpsimd.collective_compute(
        "AllGather", mybir.AluOpType.bypass, replica_groups=GROUPS,
        ins=[ccek_in[:].opt()], outs=[ccek_out[:].opt()])
    nc.gpsimd.collective_compute(
        "AllGather", mybir.AluOpType.bypass, replica_groups=GROUPS,
        ins=[ccevt_in[:].opt()], outs=[ccevt_out[:].opt()])

    # inv_col = 1/sqrt(sum_sq + eps^2), wev gate row -> lhs2[:, :, 1]
    with tc.tile_pool(name="colw", bufs=1) as colw:
        sq_col = colw.tile([P, NT_KN], F32, tag="sqc")
        nc.sync.dma_start(
            sq_col[:], ccr_out[0, :].rearrange("(j p) -> p j", p=P))
        nrm = colw.tile([P, NT_KN], F32, tag="nrm")
        nc.scalar.activation(nrm[:], sq_col[:], AF.Sqrt, bias=epsb_p[:])
        nc.vector.reciprocal(inv_col[:], nrm[:])
        wev_col = colw.tile([P, NT_KN], F32, tag="wvc")
        nc.sync.dma_start(
            wev_col[:], ccr_out[1, :].rearrange("(j p) -> p j", p=P))
        nc.any.memset(lhs2[:], 1.0)
        nc.vector.tensor_copy(lhs2[:, :, 1], wev_col[:])

    # ============ Phase KV: normalized kT, relu(vT) ==================
    kT_all, f_kT = tc.tile([P, NT_C, BL], F32, name="kT_all")
    sqk_all, f_sqk = tc.tile([P, NT_C, BL], BF16, name="sqk_all")
    xp_all, f_xp = tc.tile([P, NT_I, BL], BF16, name="xp_all")
    for i in range(NT_I):
        nc.sync.dma_start(xp_all[:, i, :], xT[i * P:(i + 1) * P, :])

    with tc.tile_pool(name="wkvp", bufs=2) as wkvp, \
         tc.tile_pool(name="pkv", bufs=2, space="PSUM") as pkv:
        for mg in range(4):
            kv_ps = [pkv.tile([P, BL], F32, tag=f"kvps{q}",
                              name=f"kvps{mg}_{q}")
                     for q in range(4)]
            for i in range(NT_I):
                wp = wkvp.tile([P, 512], BF16, tag="wp")
                nc.sync.dma_start(
                    wp[:], wkvT[i * P:(i + 1) * P,
                                mg * 512:(mg + 1) * 512])
                for q in range(4):
                    nc.tensor.matmul(
                        kv_ps[q], wp[:, q * P:(q + 1) * P],
                        xp_all[:, i, :],
                        start=(i == 0), stop=(i == NT_I - 1))
            for q in range(4):
                m = mg * 4 + q
                if m < NT_C:
                    nc.scalar.activation(
                        kT_all[:, m, :], kv_ps[q], AF.Identity,
                        bias=bkv_sb[:, m:m + 1])
                    nc.scalar.activation(
                        sqk_all[:, m, :], kv_ps[q], AF.Square,
                        bias=bkv_sb[:, m:m + 1])
                else:
                    nc.scalar.activation(
                        vr_all[:, m - NT_C, :], kv_ps[q], AF.Relu,
                        bias=bkv_sb[:, m:m + 1])
    f_xp()

    with tc.tile_pool(name="kvw", bufs=2) as kvw, \
         tc.tile_pool(name="pssk", bufs=1, space="PSUM") as pssk, \
         tc.tile_pool(name="pbc", bufs=1, space="PSUM") as pbc:
        ssk = pssk.tile([1, BL], F32)
        for m in range(NT_C):
            nc.tensor.matmul(ssk[:], ones_col[:], sqk_all[:, m, :],
                             start=(m == 0), stop=(m == NT_C - 1))
        nk = kvw.tile([1, BL], F32, tag="nk")
        nc.scalar.activation(nk[:], ssk[:], AF.Sqrt, bias=epsb[:])
        invk = kvw.tile([1, BL], F32, tag="invk")
        nc.vector.reciprocal(invk[:], nk[:])
        bc = pbc.tile([P, BL], F32)
        nc.tensor.matmul(bc[:], ones_row[:], invk[:])
        for m in range(NT_C):
            nc.vector.tensor_mul(kn_all[:, m, :], kT_all[:, m, :],
                                 bc[:])
    f_sqk()
    f_kT()

    # ============ Fused SIM + GATE + WF ==============================
    with tc.tile_pool(name="ekp", bufs=2) as ekp, \
         tc.tile_pool(name="gw", bufs=2) as gw, \
         tc.tile_pool(name="esw", bufs=8) as esw, \
         tc.tile_pool(name="psim", bufs=3, space="PSUM") as psim, \
         tc.tile_pool(name="pg", bufs=1, space="PSUM") as pg, \
         tc.tile_pool(name="pbc2", bufs=2, space="PSUM") as pbc2:
        for k in range(K):
            ks = slice(k * 512, (k + 1) * 512)
            ekt = ekp.tile([P, NT_C, 512], BF16, tag="ek")
            for m in range(NT_C):
                nc.sync.dma_start(ekt[:, m, :], ccek_out[m][:, ks])
            gse = pg.tile([1, BL], F32, tag="gse")
            gtg = pg.tile([1, BL], F32, tag="gtg")
            es_list = []
            for j in range(TPK):
                kt = k * TPK + j
                ps = psim.tile([P, BL], F32, tag="simps")
                for m in range(NT_C):
                    nc.tensor.matmul(
                        ps[:], ekt[:, m, j * P:(j + 1) * P],
                        kn_all[:, m, :],
                        start=(m == 0), stop=(m == NT_C - 1))
                es = esw.tile([P, BL], BF16, tag="esw")
                nc.scalar.activation(es[:], ps[:], AF.Exp,
                                     scale=inv_col[:, kt:kt + 1])
                es_list.append(es)
                nc.tensor.matmul(gse[:], lhs2[:, kt, 0:1], es[:],
                                 start=(j == 0), stop=(j == TPK - 1))
                nc.tensor.matmul(gtg[:], lhs2[:, kt, 1:2], es[:],
                                 start=(j == 0), stop=(j == TPK - 1))
            rs = gw.tile([1, BL], F32, tag="rs")
            nc.vector.reciprocal(rs[:], gse[:])
            tg = gw.tile([1, BL], F32, tag="tg")
            nc.vector.tensor_mul(tg[:], gtg[:], rs[:])
            fwk = gw.tile([1, BL], F32, tag="fwk")
            nc.scalar.activation(fwk[:], tg[:], AF.Sigmoid,
                                 bias=bw_sb[0:1, 0:1])
            sk = gw.tile([1, BL], F32, tag="sk")
            nc.vector.tensor_mul(sk[:], fwk[:], rs[:])
            bcs = pbc2.tile([P, BL], F32, tag="bcs")
            nc.tensor.matmul(bcs[:], ones_row[:], sk[:])
            bcs_sb = gw.tile([P, BL], BF16, tag="bcssb")
            nc.scalar.copy(bcs_sb[:], bcs[:])
            for j in range(TPK):
                kt = k * TPK + j
                nc.vector.tensor_mul(wf_all[:, kt, :], es_list[j],
                                     bcs_sb[:])
    f_kn()
    f_inv()
    f_lhs2()

    # ============ Phase FE ===========================================
    with tc.tile_pool(name="evp", bufs=2) as evp, \
         tc.tile_pool(name="pfe", bufs=3, space="PSUM") as pfe:
        for mc in range(NT_C):
            evtt = evp.tile([P, KN], BF16, tag="evt")
            nc.sync.dma_start(evtt[:], ccevt_out[mc])
            ps = pfe.tile([P, BL], F32, tag="feps")
            for kt in range(NT_KN):
                nc.tensor.matmul(
                    ps[:], evtt[:, kt * P:(kt + 1) * P],
                    wf_all[:, kt, :],
                    start=(kt == 0), stop=(kt == NT_KN - 1))
            nc.scalar.activation(fr_all[:, mc, :], ps[:], AF.Relu)
    f_wf()

    # ============ Phase OUT ==========================================
    with tc.tile_pool(name="ow", bufs=1) as ow, \
         tc.tile_pool(name="pout", bufs=1, space="PSUM") as pout:
        po = pout.tile([K, BL], F32)
        for j in range(NT_KV):
            rhs = vr_all[:, j, :] if j < NT_C else \
                fr_all[:, j - NT_C, :]
            nc.tensor.matmul(po[:], wo_sb[:, j * K:(j + 1) * K], rhs,
                             start=(j == 0), stop=(j == NT_KV - 1))
        osb = ow.tile([K, BL], F32)
        nc.scalar.activation(osb[:], po[:], AF.Identity,
                             bias=bout_sb[:])
        nc.sync.dma_start(outT[:], osb[:])
    f_fr()
    f_vr()
    _f6()
    _f5()
    _f4()
    _f3()
    _f2()
    _f1()
    _f0b()
    _f0()

    tc_cm.__exit__(None, None, None)
    nc.compile()
    return nc


def _host_prep(inputs):
    bf = ml_dtypes.bfloat16
    x_last = np.asarray(inputs["x"])[:, -1, :]  # [B, CH] f32
    wekT = np.ascontiguousarray(np.asarray(inputs["WEk"]).T)  # [CH, C]
    wevT = np.ascontiguousarray(np.asarray(inputs["WEv"]).T)
    shared = {
        "wkvT": np.ascontiguousarray(
            np.concatenate([inputs["Wk"], inputs["Wv"]], axis=0).T
        ).astype(bf),
        "statf": np.ascontiguousarray(
            np.asarray(inputs["static"]).transpose(1, 0, 2).reshape(CH, KN)
        ).astype(bf),
        "bkv": np.ascontiguousarray(
            np.concatenate([inputs["bk"], inputs["bv"]]).reshape(NT_KV, P).T),
        "ident": np.eye(P, dtype=bf),
        "wout": np.ascontiguousarray(
            np.asarray(inputs["Wout"]).T.reshape(NT_KV, P, K)
            .transpose(1, 0, 2).reshape(P, NT_KV * K)).astype(bf),
        "bws": np.asarray(inputs["bw"], dtype=np.float32).reshape(1, 1),
        "boutt": np.asarray(inputs["bout"], dtype=np.float32).reshape(K, 1),
    }
    in_maps = []
    for r in range(NCORES):
        cslc = slice(r * P, (r + 1) * P)
        m = dict(shared)
        m["xT"] = np.ascontiguousarray(
            x_last[r * BL:(r + 1) * BL].T).astype(bf)
        m["wekc"] = np.ascontiguousarray(wekT[:, cslc]).astype(bf)
        m["wevc"] = np.ascontiguousarray(wevT[:, cslc]).astype(bf)
        m["bekc"] = np.ascontiguousarray(
            np.asarray(inputs["bEk"], dtype=np.float32)[cslc].reshape(P, 1))
        m["bevc"] = np.ascontiguousarray(
            np.asarray(inputs["bEv"], dtype=np.float32)[cslc].reshape(P, 1))
        m["wwc"] = np.ascontiguousarray(
            np.asarray(inputs["Ww"])[0, cslc].reshape(P, 1)).astype(bf)
        in_maps.append(m)
    return in_maps


def kernel(**inputs):
    if "nc" not in _CACHE:
        _CACHE["nc"] = _build_nc()
    nc = _CACHE["nc"]
    in_maps = _host_prep(inputs)
    res = bass_utils.run_bass_kernel_spmd(
        nc, in_maps, core_ids=list(range(NCORES)), trace=False)
    out = np.concatenate(
        [res.results[r]["outT"].T for r in range(NCORES)], axis=0)
    return np.ascontiguousarray(out[:, :, None], dtype=np.float32)


# revision 14
# speedup vs baseline: 1.5341x; 1.0433x over previous
"""Trainium2 Bass kernel for nn_Colar_static (retrieval_knn).

Strategy: data-parallel over batch B across 8 cores PLUS tensor-parallel
split of the Ek/Ev prototype projections over the C=1024 channel dim
(each core computes a [128, K*N] slab = 1/8 of the work the baseline
replicated). Slabs are exchanged with on-chip collectives:
  - AllReduce  [2, KN] f32   : Ek column sum-of-squares + wEv gate row
  - AllGather  [128, KN] bf16: Ek slab (c-tile per rank)
  - AllGather  [128, KN] bf16: Ev^T slab (kn on partitions, c-slice free)
Collectives overlap the batch-local k/v projection phase.

DMA descriptor *triggers* cost ~607ns each, serialized on the issuing
engine (SP or Activation are the only HW-DGE triggers). So all bulk
traffic uses host-retiled [128, i, n] layouts so each logical load is
ONE multi-dim DMA, and triggers are spread across the sync and scalar
queues. KV accumulates each PSUM bank to completion (q-outer) to avoid
the bank-cycling micro-idle penalty.

SBUF singles are created in reverse order of death (LIFO pool stack).
"""

import sys

for _p in ("/opt/trn_rl_repo", "/opt/pypackages"):
    if _p not in sys.path:
        sys.path.append(_p)

import numpy as np
import ml_dtypes

import concourse.bass as bass
import concourse.mybir as mybir
import concourse.tile as tile
from concourse import bacc
from concourse import bass_utils

B, T, CH, C, N, K = 4096, 8, 2048, 1024, 512, 5
NCORES = 8
BL = B // NCORES            # 512 batch rows per core
KN = K * N                  # 2560 prototype columns
P = 128
NT_I = CH // P              # 16 contraction tiles (input channels)
NT_C = C // P               # 8 tiles over C
NT_KN = KN // P             # 20 tiles over K*N
NT_KV = 2 * C // P          # 16 tiles over [k|v] output channels
TPK = NT_KN // K            # 4 kn-tiles per prototype
NCH = KN // 512             # 5 column chunks for the slab projections
EPS = 1e-8

F32 = mybir.dt.float32
BF16 = mybir.dt.bfloat16
AF = mybir.ActivationFunctionType

_CACHE = {}


def _build_nc():
    nc = bacc.Bacc(None, target_bir_lowering=False, debug=False)

    # [P, NT_I, n] host-retiled layouts: row (i*P + p) -> [p, i]
    xTt = nc.dram_tensor("xTt", [P, NT_I, BL], BF16, kind="ExternalInput")
    wkvt = nc.dram_tensor("wkvt", [P, NT_I, 2 * C], BF16,
                          kind="ExternalInput")
    wekt = nc.dram_tensor("wekt", [P, NT_I, P], BF16, kind="ExternalInput")
    wevt = nc.dram_tensor("wevt", [P, NT_I, P], BF16, kind="ExternalInput")
    statt = nc.dram_tensor("statt", [P, NT_I, KN], BF16,
                           kind="ExternalInput")
    bekc = nc.dram_tensor("bekc", [P, 1], F32, kind="ExternalInput")
    bevc = nc.dram_tensor("bevc", [P, 1], F32, kind="ExternalInput")
    wwc = nc.dram_tensor("wwc", [P, 1], BF16, kind="ExternalInput")
    bkv = nc.dram_tensor("bkv", [P, NT_KV], F32, kind="ExternalInput")
    ident = nc.dram_tensor("ident", [P, P], BF16, kind="ExternalInput")
    wout = nc.dram_tensor("wout", [P, NT_KV * K], BF16, kind="ExternalInput")
    bws = nc.dram_tensor("bws", [1, 1], F32, kind="ExternalInput")
    boutt = nc.dram_tensor("boutt", [K, 1], F32, kind="ExternalInput")
    outT = nc.dram_tensor("outT", [K, BL], F32, kind="ExternalOutput")

    # collective bounce buffers (inputs must be Internal-Local; gather
    # outputs Shared so ranks deposit slices into one HBM buffer)
    ccr_in = nc.dram_tensor("ccr_in", [2, KN], F32)
    ccr_out = nc.dram_tensor("ccr_out", [2, KN], F32)
    ccek_in = nc.dram_tensor("ccek_in", [P, KN], BF16)
    ccek_out = nc.dram_tensor("ccek_out", [NCORES, P, KN], BF16,
                              addr_space="Shared")
    ccevt_in = nc.dram_tensor("ccevt_in", [P, KN], BF16)
    ccevt_out = nc.dram_tensor("ccevt_out", [NCORES, P, KN], BF16,
                               addr_space="Shared")
    GROUPS = [list(range(NCORES))]

    tc_cm = tile.TileContext(nc)
    tc = tc_cm.__enter__()

    # ---- engine warmups: first use of an ACT table stalls; issue tiny
    # activations up front so table loads overlap the initial DMAs.
    warm, f_warm = tc.tile([1, 16], F32, name="warm")
    nc.vector.memset(warm[:], 1.0)
    for wf_i, wfunc in enumerate((AF.Identity, AF.Square, AF.Relu, AF.Exp,
                                  AF.Sqrt, AF.Sigmoid)):
        wo_t, f_wo_t = tc.tile([1, 16], F32, name=f"warmo{wf_i}")
        nc.scalar.activation(wo_t[:], warm[:], wfunc)
        f_wo_t()
    f_warm()

    # ---- persistents (die at the very end), bottom of pool stack
    epsb, _f0 = tc.tile([1, 1], F32, name="epsb")
    nc.vector.memset(epsb[:], EPS * EPS)
    epsb_p, _f0b = tc.tile([P, 1], F32, name="epsb_p")
    nc.vector.memset(epsb_p[:], EPS * EPS)
    ones_col, _f1 = tc.tile([P, 1], BF16, name="ones_col")
    nc.any.memset(ones_col[:], 1.0)
    ones_row, _f2 = tc.tile([1, P], F32, name="ones_row")
    nc.any.memset(ones_row[:], 1.0)
    bkv_sb, _f3 = tc.tile([P, NT_KV], F32, name="bkv_sb")
    nc.sync.dma_start(bkv_sb[:], bkv[:])
    bw_sb, _f4 = tc.tile([1, 1], F32, name="bw_sb")
    nc.sync.dma_start(bw_sb[:], bws[:])
    bout_sb, _f5 = tc.tile([K, 1], F32, name="bout_sb")
    nc.sync.dma_start(bout_sb[:], boutt[:])
    wo_sb, _f6 = tc.tile([P, NT_KV * K], BF16, name="wo_sb")
    nc.sync.dma_start(wo_sb[:], wout[:])

    # dies OUT-end
    vr_all, f_vr = tc.tile([P, NT_C, BL], BF16, name="vr_all")
    fr_all, f_fr = tc.tile([P, NT_C, BL], BF16, name="fr_all")
    # dies FE-end
    wf_all, f_wf = tc.tile([P, NT_KN, BL], BF16, name="wf_all")
    # die SIM-end (written after collectives)
    lhs2, f_lhs2 = tc.tile([P, NT_KN, 2], BF16, name="lhs2")
    inv_col, f_inv = tc.tile([P, NT_KN], F32, name="inv_col")
    kn_all, f_kn = tc.tile([P, NT_C, BL], BF16, name="kn_all")
    # die KV-end
    kT_all, f_kT = tc.tile([P, NT_C, BL], BF16, name="kT_all")
    sqk_all, f_sqk = tc.tile([P, NT_C, BL], BF16, name="sqk_all")
    xp_all, f_xp = tc.tile([P, NT_I, BL], BF16, name="xp_all")
    nc.sync.dma_start(xp_all[:], xTt[:])
    wblk0, f_wblk0 = tc.tile([P, NT_I, 512], BF16, name="wblk0")
    nc.scalar.dma_start(wblk0[:], wkvt[:, :, 0:512])

    # ============ Phase P: Ek / Ev^T slabs (this core's 128 c's) =====
    wek_sb, f_wek = tc.tile([P, NT_I, P], BF16, name="wek_sb")
    nc.scalar.dma_start(wek_sb[:], wekt[:])
    wev_sb, f_wev = tc.tile([P, NT_I, P], BF16, name="wev_sb")
    nc.scalar.dma_start(wev_sb[:], wevt[:])
    bekc_sb, f_bek = tc.tile([P, 1], F32, name="bekc_sb")
    nc.scalar.dma_start(bekc_sb[:], bekc[:])
    bevc_sb, f_bev = tc.tile([P, 1], F32, name="bevc_sb")
    nc.scalar.dma_start(bevc_sb[:], bevc[:])
    wwc_sb, f_ww = tc.tile([P, 1], BF16, name="wwc_sb")
    nc.scalar.dma_start(wwc_sb[:], wwc[:])
    id_sb, f_id = tc.tile([P, P], BF16, name="id_sb")
    nc.scalar.dma_start(id_sb[:], ident[:])
    ek_slab, f_eks = tc.tile([P, KN], BF16, name="ek_slab")
    evt_slab, f_evs = tc.tile([P, KN], BF16, name="evt_slab")

    with tc.tile_pool(name="stp", bufs=2) as stp, \
         tc.tile_pool(name="pw", bufs=3) as pw, \
         tc.tile_pool(name="rowp", bufs=2) as rowp, \
         tc.tile_pool(name="ppk", bufs=2, space="PSUM") as ppk, \
         tc.tile_pool(name="prow", bufs=1, space="PSUM") as prow, \
         tc.tile_pool(name="ptp", bufs=2, space="PSUM") as ptp:
        for ch in range(NCH):
            cs = slice(ch * 512, (ch + 1) * 512)
            stch = stp.tile([P, NT_I, 512], BF16, tag="st")
            nc.sync.dma_start(stch[:], statt[:, :, cs])
            # Ek chunk
            ek_ps = ppk.tile([P, 512], F32, tag="ek")
            for i in range(NT_I):
                nc.tensor.matmul(ek_ps[:], wek_sb[:, i, :], stch[:, i, :],
                                 start=(i == 0), stop=(i == NT_I - 1))
            nc.scalar.activation(ek_slab[:, cs], ek_ps[:], AF.Identity,
                                 bias=bekc_sb[:])
            sqt = pw.tile([P, 512], BF16, tag="sq")
            nc.scalar.activation(sqt[:], ek_ps[:], AF.Square,
                                 bias=bekc_sb[:])
            sq_ps = prow.tile([1, 512], F32, tag="row")
            nc.tensor.matmul(sq_ps[:], ones_col[:], sqt[:])
            sq_row = rowp.tile([1, 512], F32, tag="sqr")
            nc.vector.tensor_copy(sq_row[:], sq_ps[:])
            nc.scalar.dma_start(ccr_in[0:1, cs], sq_row[:])
            # Ev chunk
            ev_ps = ppk.tile([P, 512], F32, tag="ev")
            for i in range(NT_I):
                nc.tensor.matmul(ev_ps[:], wev_sb[:, i, :], stch[:, i, :],
                                 start=(i == 0), stop=(i == NT_I - 1))
            evbf = pw.tile([P, 512], BF16, tag="ev")
            nc.scalar.activation(evbf[:], ev_ps[:], AF.Identity,
                                 bias=bevc_sb[:])
            wev_ps = prow.tile([1, 512], F32, tag="row")
            nc.tensor.matmul(wev_ps[:], wwc_sb[:], evbf[:])
            wev_row = rowp.tile([1, 512], F32, tag="wvr")
            nc.vector.tensor_copy(wev_row[:], wev_ps[:])
            nc.scalar.dma_start(ccr_in[1:2, cs], wev_row[:])
            # Ev^T chunk (4 PE transposes via identity)
            tp_ps = ptp.tile([P, 512], BF16, tag="tp")
            for q in range(4):
                nc.tensor.transpose(tp_ps[:, q * P:(q + 1) * P],
                                    evbf[:, q * P:(q + 1) * P], id_sb[:])
            nc.scalar.copy(evt_slab[:, cs], tp_ps[:])
        nc.sync.dma_start(ccek_in[:], ek_slab[:])
        nc.sync.dma_start(ccevt_in[:], evt_slab[:])
    f_evs()
    f_eks()
    f_id()
    f_ww()
    f_bev()
    f_bek()
    f_wev()
    f_wek()

    # ============ Collectives (overlap the KV phase) ==================
    nc.gpsimd.collective_compute(
        "AllReduce", mybir.AluOpType.add, replica_groups=GROUPS,
        ins=[ccr_in[:].opt()], outs=[ccr_out[:].opt()])
    nc.gpsimd.collective_compute(
        "AllGather", mybir.AluOpType.bypass, replica_groups=GROUPS,
        ins=[ccek_in[:].opt()], outs=[ccek_out[:].opt()])
    nc.gpsimd.collective_compute(
        "AllGather", mybir.AluOpType.bypass, replica_groups=GROUPS,
        ins=[ccevt_in[:].opt()], outs=[ccevt_out[:].opt()])

    # inv_col = 1/sqrt(sum_sq + eps^2); wEv gate row -> lhs2[:, :, 1]
    with tc.tile_pool(name="colw", bufs=1) as colw:
        sq_col = colw.tile([P, NT_KN], F32, tag="sqc")
        nc.scalar.dma_start(
            sq_col[:], ccr_out[0, :].rearrange("(j p) -> p j", p=P))
        nrm = colw.tile([P, NT_KN], F32, tag="nrm")
        nc.scalar.activation(nrm[:], sq_col[:], AF.Sqrt, bias=epsb_p[:])
        nc.vector.reciprocal(inv_col[:], nrm[:])
        wev_col = colw.tile([P, NT_KN], F32, tag="wvc")
        nc.scalar.dma_start(
            wev_col[:], ccr_out[1, :].rearrange("(j p) -> p j", p=P))
        nc.any.memset(lhs2[:], 1.0)
        nc.vector.tensor_copy(lhs2[:, :, 1], wev_col[:])

    # ============ Phase KV: normalized kT, relu(vT) ==================
    # q-outer so each PSUM bank accumulates its 16 steps back-to-back.
    with tc.tile_pool(name="wkvp", bufs=2) as wkvp, \
         tc.tile_pool(name="pkv", bufs=4, space="PSUM") as pkv:
        for mg in range(4):
            if mg == 0:
                wblk = wblk0
            else:
                wblk = wkvp.tile([P, NT_I, 512], BF16, tag="wb")
                eng = nc.sync if mg % 2 else nc.scalar
                eng.dma_start(wblk[:],
                              wkvt[:, :, mg * 512:(mg + 1) * 512])
            for q in range(4):
                m = mg * 4 + q
                kv_ps = pkv.tile([P, BL], F32, tag="kv", name=f"kv{m}")
                for i in range(NT_I):
                    nc.tensor.matmul(
                        kv_ps[:], wblk[:, i, q * P:(q + 1) * P],
                        xp_all[:, i, :],
                        start=(i == 0), stop=(i == NT_I - 1))
                if m < NT_C:
                    nc.scalar.activation(
                        kT_all[:, m, :], kv_ps[:], AF.Identity,
                        bias=bkv_sb[:, m:m + 1])
                    nc.scalar.activation(
                        sqk_all[:, m, :], kv_ps[:], AF.Square,
                        bias=bkv_sb[:, m:m + 1])
                else:
                    nc.scalar.activation(
                        vr_all[:, m - NT_C, :], kv_ps[:], AF.Relu,
                        bias=bkv_sb[:, m:m + 1])

    with tc.tile_pool(name="kvw", bufs=2) as kvw, \
         tc.tile_pool(name="pssk", bufs=1, space="PSUM") as pssk, \
         tc.tile_pool(name="pbc", bufs=1, space="PSUM") as pbc:
        ssk = pssk.tile([1, BL], F32)
        for m in range(NT_C):
            nc.tensor.matmul(ssk[:], ones_col[:], sqk_all[:, m, :],
                             start=(m == 0), stop=(m == NT_C - 1))
        nk = kvw.tile([1, BL], F32, tag="nk")
        nc.scalar.activation(nk[:], ssk[:], AF.Sqrt, bias=epsb[:])
        invk = kvw.tile([1, BL], F32, tag="invk")
        nc.vector.reciprocal(invk[:], nk[:])
        bc = pbc.tile([P, BL], F32)
        nc.tensor.matmul(bc[:], ones_row[:], invk[:])
        for m in range(NT_C):
            nc.vector.tensor_mul(kn_all[:, m, :], kT_all[:, m, :],
                                 bc[:])
    f_wblk0()
    f_xp()
    f_sqk()
    f_kT()

    # ============ Fused SIM + GATE + WF ==============================
    with tc.tile_pool(name="ekp", bufs=2) as ekp, \
         tc.tile_pool(name="gw", bufs=2) as gw, \
         tc.tile_pool(name="esw", bufs=8) as esw, \
         tc.tile_pool(name="psim", bufs=3, space="PSUM") as psim, \
         tc.tile_pool(name="pg", bufs=1, space="PSUM") as pg, \
         tc.tile_pool(name="pbc2", bufs=2, space="PSUM") as pbc2:
        for k in range(K):
            ks = slice(k * 512, (k + 1) * 512)
            ekt = ekp.tile([P, NT_C, 512], BF16, tag="ek")
            eng = nc.sync if k % 2 else nc.scalar
            eng.dma_start(ekt[:],
                          ccek_out[:, :, ks].rearrange("m p c -> p m c"))
            gse = pg.tile([1, BL], F32, tag="gse")
            gtg = pg.tile([1, BL], F32, tag="gtg")
            es_list = []
            for j in range(TPK):
                kt = k * TPK + j
                ps = psim.tile([P, BL], F32, tag="simps")
                for m in range(NT_C):
                    nc.tensor.matmul(
                        ps[:], ekt[:, m, j * P:(j + 1) * P],
                        kn_all[:, m, :],
                        start=(m == 0), stop=(m == NT_C - 1))
                es = esw.tile([P, BL], BF16, tag="esw")
                nc.scalar.activation(es[:], ps[:], AF.Exp,
                                     scale=inv_col[:, kt:kt + 1])
                es_list.append(es)
                nc.tensor.matmul(gse[:], lhs2[:, kt, 0:1], es[:],
                                 start=(j == 0), stop=(j == TPK - 1))
                nc.tensor.matmul(gtg[:], lhs2[:, kt, 1:2], es[:],
                                 start=(j == 0), stop=(j == TPK - 1))
            rs = gw.tile([1, BL], F32, tag="rs")
            nc.vector.reciprocal(rs[:], gse[:])
            tg = gw.tile([1, BL], F32, tag="tg")
            nc.vector.tensor_mul(tg[:], gtg[:], rs[:])
            fwk = gw.tile([1, BL], F32, tag="fwk")
            nc.scalar.activation(fwk[:], tg[:], AF.Sigmoid,
                                 bias=bw_sb[0:1, 0:1])
            sk = gw.tile([1, BL], F32, tag="sk")
            nc.vector.tensor_mul(sk[:], fwk[:], rs[:])
            bcs = pbc2.tile([P, BL], F32, tag="bcs")
            nc.tensor.matmul(bcs[:], ones_row[:], sk[:])
            bcs_sb = gw.tile([P, BL], BF16, tag="bcssb")
            nc.scalar.copy(bcs_sb[:], bcs[:])
            for j in range(TPK):
                kt = k * TPK + j
                nc.vector.tensor_mul(wf_all[:, kt, :], es_list[j],
                                     bcs_sb[:])
    f_kn()
    f_inv()
    f_lhs2()

    # ============ Phase FE ===========================================
    with tc.tile_pool(name="evp", bufs=2) as evp, \
         tc.tile_pool(name="pfe", bufs=3, space="PSUM") as pfe:
        for mc in range(NT_C):
            evtt = evp.tile([P, KN], BF16, tag="evt")
            eng = nc.sync if mc % 2 else nc.scalar
            eng.dma_start(evtt[:], ccevt_out[mc])
            ps = pfe.tile([P, BL], F32, tag="feps")
            for kt in range(NT_KN):
                nc.tensor.matmul(
                    ps[:], evtt[:, kt * P:(kt + 1) * P],
                    wf_all[:, kt, :],
                    start=(kt == 0), stop=(kt == NT_KN - 1))
            nc.scalar.activation(fr_all[:, mc, :], ps[:], AF.Relu)
    f_wf()

    # ============ Phase OUT ==========================================
    with tc.tile_pool(name="ow", bufs=1) as ow, \
         tc.tile_pool(name="pout", bufs=1, space="PSUM") as pout:
        po = pout.tile([K, BL], F32)
        for j in range(NT_KV):
            rhs = vr_all[:, j, :] if j < NT_C else \
                fr_all[:, j - NT_C, :]
            nc.tensor.matmul(po[:], wo_sb[:, j * K:(j + 1) * K], rhs,
                             start=(j == 0), stop=(j == NT_KV - 1))
        osb = ow.tile([K, BL], F32)
        nc.scalar.activation(osb[:], po[:], AF.Identity,
                             bias=bout_sb[:])
        nc.sync.dma_start(outT[:], osb[:])
    f_fr()
    f_vr()
    _f6()
    _f5()
    _f4()
    _f3()
    _f2()
    _f1()
    _f0b()
    _f0()

    tc_cm.__exit__(None, None, None)
    nc.compile()
    return nc


def _tile_rows(a):
    """[NT_I*P, n] -> [P, NT_I, n]: row (i*P + p) -> [p, i]."""
    n = a.shape[1]
    return np.ascontiguousarray(
        a.reshape(NT_I, P, n).transpose(1, 0, 2))


def _host_prep(inputs):
    bf = ml_dtypes.bfloat16
    x_last = np.asarray(inputs["x"])[:, -1, :]  # [B, CH] f32
    wekT = np.asarray(inputs["WEk"]).T  # [CH, C]
    wevT = np.asarray(inputs["WEv"]).T
    shared = {
        "wkvt": _tile_rows(
            np.concatenate([inputs["Wk"], inputs["Wv"]], axis=0).T
        ).astype(bf),
        "statt": _tile_rows(
            np.asarray(inputs["static"]).transpose(1, 0, 2).reshape(CH, KN)
        ).astype(bf),
        "bkv": np.ascontiguousarray(
            np.concatenate([inputs["bk"], inputs["bv"]]).reshape(NT_KV, P).T),
        "ident": np.eye(P, dtype=bf),
        "wout": np.ascontiguousarray(
            np.asarray(inputs["Wout"]).T.reshape(NT_KV, P, K)
            .transpose(1, 0, 2).reshape(P, NT_KV * K)).astype(bf),
        "bws": np.asarray(inputs["bw"], dtype=np.float32).reshape(1, 1),
        "boutt": np.asarray(inputs["bout"], dtype=np.float32).reshape(K, 1),
    }
    in_maps = []
    for r in range(NCORES):
        cslc = slice(r * P, (r + 1) * P)
        m = dict(shared)
        m["xTt"] = _tile_rows(
            np.ascontiguousarray(x_last[r * BL:(r + 1) * BL].T)).astype(bf)
        m["wekt"] = _tile_rows(
            np.ascontiguousarray(wekT[:, cslc])).astype(bf)
        m["wevt"] = _tile_rows(
            np.ascontiguousarray(wevT[:, cslc])).astype(bf)
        m["bekc"] = np.ascontiguousarray(
            np.asarray(inputs["bEk"], dtype=np.float32)[cslc].reshape(P, 1))
        m["bevc"] = np.ascontiguousarray(
            np.asarray(inputs["bEv"], dtype=np.float32)[cslc].reshape(P, 1))
        m["wwc"] = np.ascontiguousarray(
            np.asarray(inputs["Ww"])[0, cslc].reshape(P, 1)).astype(bf)
        in_maps.append(m)
    return in_maps


def kernel(**inputs):
    if "nc" not in _CACHE:
        _CACHE["nc"] = _build_nc()
    nc = _CACHE["nc"]
    in_maps = _host_prep(inputs)
    res = bass_utils.run_bass_kernel_spmd(
        nc, in_maps, core_ids=list(range(NCORES)), trace=False)
    out = np.concatenate(
        [res.results[r]["outT"].T for r in range(NCORES)], axis=0)
    return np.ascontiguousarray(out[:, :, None], dtype=np.float32)


# revision 22
# speedup vs baseline: 1.6815x; 1.0961x over previous
"""Trainium2 Bass kernel for nn_Colar_static (retrieval_knn).

Strategy: data-parallel over batch B across 8 cores PLUS tensor-parallel
split of the Ek/Ev prototype projections over the C=1024 channel dim
(each core computes a [128, K*N] slab = 1/8 of the work the baseline
replicated). Slabs are exchanged with on-chip collectives:
  - AllReduce  [2, KN] f32   : Ek column sum-of-squares + wEv gate row
  - AllGather  [128, KN] bf16: Ek slab (c-tile per rank)
  - AllGather  [128, KN] bf16: Ev^T slab (kn on partitions, c-slice free)
Collectives overlap the batch-local k/v projection phase.

DMA descriptor *triggers* cost ~607ns each, serialized on the issuing
engine (SP or Activation are the only HW-DGE triggers). So all bulk
traffic uses host-retiled [128, i, n] layouts so each logical load is
ONE multi-dim DMA, and triggers are spread across the sync and scalar
queues. KV accumulates each PSUM bank to completion (q-outer) to avoid
the bank-cycling micro-idle penalty.

SBUF singles are created in reverse order of death (LIFO pool stack).
"""

import sys

for _p in ("/opt/trn_rl_repo", "/opt/pypackages"):
    if _p not in sys.path:
        sys.path.append(_p)

import numpy as np
import ml_dtypes

import concourse.bass as bass
import concourse.mybir as mybir
import concourse.tile as tile
from concourse import bacc
from concourse import bass_utils

B, T, CH, C, N, K = 4096, 8, 2048, 1024, 512, 5
NCORES = 8
BL = B // NCORES            # 512 batch rows per core
KN = K * N                  # 2560 prototype columns
P = 128
NT_I = CH // P              # 16 contraction tiles (input channels)
NT_C = C // P               # 8 tiles over C
NT_KN = KN // P             # 20 tiles over K*N
NT_KV = 2 * C // P          # 16 tiles over [k|v] output channels
TPK = NT_KN // K            # 4 kn-tiles per prototype
NCH = KN // 512             # 5 column chunks for the slab projections
EPS = 1e-8

F32 = mybir.dt.float32
BF16 = mybir.dt.bfloat16
AF = mybir.ActivationFunctionType

_CACHE = {}


def _build_nc():
    nc = bacc.Bacc(None, target_bir_lowering=False, debug=False)

    # [P, NT_I, n] host-retiled layouts: row (i*P + p) -> [p, i]
    xTt = nc.dram_tensor("xTt", [P, NT_I, BL], BF16, kind="ExternalInput")
    wkvt = nc.dram_tensor("wkvt", [P, NT_I, 2 * C], BF16,
                          kind="ExternalInput")
    wekt = nc.dram_tensor("wekt", [P, NT_I, P], BF16, kind="ExternalInput")
    wevt = nc.dram_tensor("wevt", [P, NT_I, P], BF16, kind="ExternalInput")
    statt = nc.dram_tensor("statt", [P, NT_I, KN], BF16,
                           kind="ExternalInput")
    bekc = nc.dram_tensor("bekc", [P, 1], F32, kind="ExternalInput")
    bevc = nc.dram_tensor("bevc", [P, 1], F32, kind="ExternalInput")
    wwc = nc.dram_tensor("wwc", [P, 1], BF16, kind="ExternalInput")
    bkv = nc.dram_tensor("bkv", [P, NT_KV], F32, kind="ExternalInput")
    ident = nc.dram_tensor("ident", [P, P], BF16, kind="ExternalInput")
    wout = nc.dram_tensor("wout", [P, NT_KV * K], BF16, kind="ExternalInput")
    bws = nc.dram_tensor("bws", [1, 1], F32, kind="ExternalInput")
    boutt = nc.dram_tensor("boutt", [K, 1], F32, kind="ExternalInput")
    outT = nc.dram_tensor("outT", [K, BL], F32, kind="ExternalOutput")

    # collective bounce buffers (inputs must be Internal-Local; gather
    # outputs Shared so ranks deposit slices into one HBM buffer)
    ccr_in = nc.dram_tensor("ccr_in", [2, KN], F32)
    ccr_out = nc.dram_tensor("ccr_out", [2, KN], F32)
    ccek_in = nc.dram_tensor("ccek_in", [P, KN], BF16)
    ccek_out = nc.dram_tensor("ccek_out", [NCORES, P, KN], BF16,
                              addr_space="Shared")
    ccevt_in = nc.dram_tensor("ccevt_in", [P, KN], BF16)
    ccevt_out = nc.dram_tensor("ccevt_out", [NCORES, P, KN], BF16,
                               addr_space="Shared")
    GROUPS = [list(range(NCORES))]

    tc_cm = tile.TileContext(nc)
    tc = tc_cm.__enter__()

    # ---- engine warmups: first use of an ACT table stalls; issue tiny
    # activations up front so table loads overlap the initial DMAs.
    warm, f_warm = tc.tile([1, 16], F32, name="warm")
    nc.vector.memset(warm[:], 1.0)
    for wf_i, wfunc in enumerate((AF.Identity, AF.Square, AF.Relu, AF.Exp,
                                  AF.Sqrt, AF.Sigmoid)):
        wo_t, f_wo_t = tc.tile([1, 16], F32, name=f"warmo{wf_i}")
        nc.scalar.activation(wo_t[:], warm[:], wfunc)
        f_wo_t()
    f_warm()

    # ---- persistents (die at the very end), bottom of pool stack
    epsb, _f0 = tc.tile([1, 1], F32, name="epsb")
    nc.vector.memset(epsb[:], EPS * EPS)
    epsb_p, _f0b = tc.tile([P, 1], F32, name="epsb_p")
    nc.vector.memset(epsb_p[:], EPS * EPS)
    ones_col, _f1 = tc.tile([P, 1], BF16, name="ones_col")
    nc.any.memset(ones_col[:], 1.0)
    ones_row, _f2 = tc.tile([1, P], F32, name="ones_row")
    nc.any.memset(ones_row[:], 1.0)
    bkv_sb, _f3 = tc.tile([P, NT_KV], F32, name="bkv_sb")
    nc.sync.dma_start(bkv_sb[:], bkv[:])
    bw_sb, _f4 = tc.tile([1, 1], F32, name="bw_sb")
    nc.sync.dma_start(bw_sb[:], bws[:])
    bout_sb, _f5 = tc.tile([K, 1], F32, name="bout_sb")
    nc.sync.dma_start(bout_sb[:], boutt[:])
    wo_sb, _f6 = tc.tile([P, NT_KV * K], BF16, name="wo_sb")
    nc.sync.dma_start(wo_sb[:], wout[:])

    # dies OUT-end
    vr_all, f_vr = tc.tile([P, NT_C, BL], BF16, name="vr_all")
    fr_all, f_fr = tc.tile([P, NT_C, BL], BF16, name="fr_all")
    # dies FE-end
    wf_all, f_wf = tc.tile([P, NT_KN, BL], BF16, name="wf_all")
    # die SIM-end (written after collectives)
    lhs2, f_lhs2 = tc.tile([P, NT_KN, 2], BF16, name="lhs2")
    inv_col, f_inv = tc.tile([P, NT_KN], F32, name="inv_col")
    kn_all, f_kn = tc.tile([P, NT_C, BL], BF16, name="kn_all")
    # die KV-end
    kT_all, f_kT = tc.tile([P, NT_C, BL], BF16, name="kT_all")
    sqk_all, f_sqk = tc.tile([P, NT_C, BL], BF16, name="sqk_all")
    xp_all, f_xp = tc.tile([P, NT_I, BL], BF16, name="xp_all")
    nc.sync.dma_start(xp_all[:], xTt[:])
    wblk0, f_wblk0 = tc.tile([P, NT_I, 512], BF16, name="wblk0")
    nc.scalar.dma_start(wblk0[:], wkvt[:, :, 0:512])
    wblk1, f_wblk1 = tc.tile([P, NT_I, 512], BF16, name="wblk1")
    nc.scalar.dma_start(wblk1[:], wkvt[:, :, 512:1024])

    # ============ Phase P: Ek / Ev^T slabs (this core's 128 c's) =====
    wek_sb, f_wek = tc.tile([P, NT_I, P], BF16, name="wek_sb")
    nc.sync.dma_start(wek_sb[:], wekt[:])
    wev_sb, f_wev = tc.tile([P, NT_I, P], BF16, name="wev_sb")
    nc.scalar.dma_start(wev_sb[:], wevt[:])
    bekc_sb, f_bek = tc.tile([P, 1], F32, name="bekc_sb")
    nc.scalar.dma_start(bekc_sb[:], bekc[:])
    bevc_sb, f_bev = tc.tile([P, 1], F32, name="bevc_sb")
    nc.scalar.dma_start(bevc_sb[:], bevc[:])
    wwc_sb, f_ww = tc.tile([P, 1], BF16, name="wwc_sb")
    nc.scalar.dma_start(wwc_sb[:], wwc[:])
    id_sb, f_id = tc.tile([P, P], BF16, name="id_sb")
    nc.scalar.dma_start(id_sb[:], ident[:])
    ek_slab, f_eks = tc.tile([P, KN], BF16, name="ek_slab")
    evt_slab, f_evs = tc.tile([P, KN], BF16, name="evt_slab")

    with tc.tile_pool(name="stp", bufs=2) as stp, \
         tc.tile_pool(name="pw", bufs=3) as pw, \
         tc.tile_pool(name="rowp", bufs=2) as rowp, \
         tc.tile_pool(name="ppk", bufs=2, space="PSUM") as ppk, \
         tc.tile_pool(name="prow", bufs=1, space="PSUM") as prow, \
         tc.tile_pool(name="ptp", bufs=2, space="PSUM") as ptp:
        for ch in range(NCH):
            cs = slice(ch * 512, (ch + 1) * 512)
            stch = stp.tile([P, NT_I, 512], BF16, tag="st")
            nc.sync.dma_start(stch[:, 0:NT_I // 2, :],
                              statt[:, 0:NT_I // 2, cs])
            nc.scalar.dma_start(stch[:, NT_I // 2:, :],
                                statt[:, NT_I // 2:, cs])
            # Ek chunk
            ek_ps = ppk.tile([P, 512], F32, tag="ek")
            for i in range(NT_I):
                nc.tensor.matmul(ek_ps[:], wek_sb[:, i, :], stch[:, i, :],
                                 start=(i == 0), stop=(i == NT_I - 1))
            nc.scalar.activation(ek_slab[:, cs], ek_ps[:], AF.Identity,
                                 bias=bekc_sb[:])
            sqt = pw.tile([P, 512], BF16, tag="sq")
            nc.scalar.activation(sqt[:], ek_ps[:], AF.Square,
                                 bias=bekc_sb[:])
            sq_ps = prow.tile([1, 512], F32, tag="row")
            nc.tensor.matmul(sq_ps[:], ones_col[:], sqt[:])
            sq_row = rowp.tile([1, 512], F32, tag="sqr")
            nc.vector.tensor_copy(sq_row[:], sq_ps[:])
            nc.scalar.dma_start(ccr_in[0:1, cs], sq_row[:])
            # Ev chunk
            ev_ps = ppk.tile([P, 512], F32, tag="ev")
            for i in range(NT_I):
                nc.tensor.matmul(ev_ps[:], wev_sb[:, i, :], stch[:, i, :],
                                 start=(i == 0), stop=(i == NT_I - 1))
            evbf = pw.tile([P, 512], BF16, tag="ev")
            nc.scalar.activation(evbf[:], ev_ps[:], AF.Identity,
                                 bias=bevc_sb[:])
            wev_ps = prow.tile([1, 512], F32, tag="row")
            nc.tensor.matmul(wev_ps[:], wwc_sb[:], evbf[:])
            wev_row = rowp.tile([1, 512], F32, tag="wvr")
            nc.vector.tensor_copy(wev_row[:], wev_ps[:])
            nc.scalar.dma_start(ccr_in[1:2, cs], wev_row[:])
            # Ev^T chunk (4 PE transposes via identity)
            tp_ps = ptp.tile([P, 512], BF16, tag="tp")
            for q in range(4):
                nc.tensor.transpose(tp_ps[:, q * P:(q + 1) * P],
                                    evbf[:, q * P:(q + 1) * P], id_sb[:])
            nc.scalar.copy(evt_slab[:, cs], tp_ps[:])
        nc.sync.dma_start(ccek_in[:], ek_slab[:])
        nc.scalar.dma_start(ccevt_in[:], evt_slab[:])
    f_evs()
    f_eks()
    f_id()
    f_ww()
    f_bev()
    f_bek()
    f_wev()
    f_wek()

    # ============ Collectives (overlap the KV phase) ==================
    nc.gpsimd.collective_compute(
        "AllReduce", mybir.AluOpType.add, replica_groups=GROUPS,
        ins=[ccr_in[:].opt()], outs=[ccr_out[:].opt()])
    nc.gpsimd.collective_compute(
        "AllGather", mybir.AluOpType.bypass, replica_groups=GROUPS,
        ins=[ccek_in[:].opt()], outs=[ccek_out[:].opt()])
    nc.gpsimd.collective_compute(
        "AllGather", mybir.AluOpType.bypass, replica_groups=GROUPS,
        ins=[ccevt_in[:].opt()], outs=[ccevt_out[:].opt()])

    # inv_col = 1/sqrt(sum_sq + eps^2); wEv gate row -> lhs2[:, :, 1]
    with tc.tile_pool(name="colw", bufs=1) as colw:
        sq_col = colw.tile([P, NT_KN], F32, tag="sqc")
        nc.scalar.dma_start(
            sq_col[:], ccr_out[0, :].rearrange("(j p) -> p j", p=P))
        nrm = colw.tile([P, NT_KN], F32, tag="nrm")
        nc.scalar.activation(nrm[:], sq_col[:], AF.Sqrt, bias=epsb_p[:])
        nc.vector.reciprocal(inv_col[:], nrm[:])
        wev_col = colw.tile([P, NT_KN], F32, tag="wvc")
        nc.scalar.dma_start(
            wev_col[:], ccr_out[1, :].rearrange("(j p) -> p j", p=P))
        nc.any.memset(lhs2[:], 1.0)
        nc.vector.tensor_copy(lhs2[:, :, 1], wev_col[:])

    # ============ Phase KV: normalized kT, relu(vT) ==================
    # q-outer so each PSUM bank accumulates its 16 steps back-to-back.
    with tc.tile_pool(name="wkvp", bufs=2) as wkvp, \
         tc.tile_pool(name="pkv", bufs=4, space="PSUM") as pkv:
        for mg in range(4):
            if mg == 0:
                wblk = wblk0
            elif mg == 1:
                wblk = wblk1
            else:
                ms = slice(mg * 512, (mg + 1) * 512)
                wblk = wkvp.tile([P, NT_I, 512], BF16, tag="wb")
                nc.sync.dma_start(wblk[:, 0:NT_I // 2, :],
                                  wkvt[:, 0:NT_I // 2, ms])
                nc.scalar.dma_start(wblk[:, NT_I // 2:, :],
                                    wkvt[:, NT_I // 2:, ms])
            for q in range(4):
                m = mg * 4 + q
                kv_ps = pkv.tile([P, BL], F32, tag="kv", name=f"kv{m}")
                for i in range(NT_I):
                    nc.tensor.matmul(
                        kv_ps[:], wblk[:, i, q * P:(q + 1) * P],
                        xp_all[:, i, :],
                        start=(i == 0), stop=(i == NT_I - 1))
                if m < NT_C:
                    nc.scalar.activation(
                        kT_all[:, m, :], kv_ps[:], AF.Identity,
                        bias=bkv_sb[:, m:m + 1])
                    nc.scalar.activation(
                        sqk_all[:, m, :], kv_ps[:], AF.Square,
                        bias=bkv_sb[:, m:m + 1])
                else:
                    nc.scalar.activation(
                        vr_all[:, m - NT_C, :], kv_ps[:], AF.Relu,
                        bias=bkv_sb[:, m:m + 1])

    with tc.tile_pool(name="kvw", bufs=2) as kvw, \
         tc.tile_pool(name="pssk", bufs=1, space="PSUM") as pssk, \
         tc.tile_pool(name="pbc", bufs=1, space="PSUM") as pbc:
        ssk = pssk.tile([1, BL], F32)
        for m in range(NT_C):
            nc.tensor.matmul(ssk[:], ones_col[:], sqk_all[:, m, :],
                             start=(m == 0), stop=(m == NT_C - 1))
        nk = kvw.tile([1, BL], F32, tag="nk")
        nc.scalar.activation(nk[:], ssk[:], AF.Sqrt, bias=epsb[:])
        invk = kvw.tile([1, BL], F32, tag="invk")
        nc.vector.reciprocal(invk[:], nk[:])
        bc = pbc.tile([P, BL], F32)
        nc.tensor.matmul(bc[:], ones_row[:], invk[:])
        for m in range(NT_C):
            nc.vector.tensor_mul(kn_all[:, m, :], kT_all[:, m, :],
                                 bc[:])
    f_wblk1()
    f_wblk0()
    f_xp()
    f_sqk()
    f_kT()

    # ============ Fused SIM + GATE + WF ==============================
    with tc.tile_pool(name="ekp", bufs=2) as ekp, \
         tc.tile_pool(name="gw", bufs=2) as gw, \
         tc.tile_pool(name="esw", bufs=8) as esw, \
         tc.tile_pool(name="psim", bufs=3, space="PSUM") as psim, \
         tc.tile_pool(name="pg", bufs=2, space="PSUM") as pg, \
         tc.tile_pool(name="pbc2", bufs=1, space="PSUM") as pbc2:
        for k in range(K):
            ks = slice(k * 512, (k + 1) * 512)
            ekt = ekp.tile([P, NT_C, 512], BF16, tag="ek")
            nc.gpsimd.dma_start(
                ekt[:], ccek_out[:, :, ks].rearrange("m p c -> p m c"))
            gse = pg.tile([1, BL], F32, tag="gse")
            gtg = pg.tile([1, BL], F32, tag="gtg")
            es_list = []
            for j in range(TPK):
                kt = k * TPK + j
                ps = psim.tile([P, BL], F32, tag="simps")
                for m in range(NT_C):
                    nc.tensor.matmul(
                        ps[:], ekt[:, m, j * P:(j + 1) * P],
                        kn_all[:, m, :],
                        start=(m == 0), stop=(m == NT_C - 1))
                es = esw.tile([P, BL], BF16, tag="esw")
                nc.scalar.activation(es[:], ps[:], AF.Exp,
                                     scale=inv_col[:, kt:kt + 1])
                es_list.append(es)
                nc.tensor.matmul(gse[:], lhs2[:, kt, 0:1], es[:],
                                 start=(j == 0), stop=(j == TPK - 1))
                nc.tensor.matmul(gtg[:], lhs2[:, kt, 1:2], es[:],
                                 start=(j == 0), stop=(j == TPK - 1))
            rs = gw.tile([1, BL], F32, tag="rs")
            nc.vector.reciprocal(rs[:], gse[:])
            tg = gw.tile([1, BL], F32, tag="tg")
            nc.vector.tensor_mul(tg[:], gtg[:], rs[:])
            fwk = gw.tile([1, BL], F32, tag="fwk")
            nc.scalar.activation(fwk[:], tg[:], AF.Sigmoid,
                                 bias=bw_sb[0:1, 0:1])
            sk = gw.tile([1, BL], F32, tag="sk")
            nc.vector.tensor_mul(sk[:], fwk[:], rs[:])
            bcs = pbc2.tile([P, BL], F32, tag="bcs")
            nc.tensor.matmul(bcs[:], ones_row[:], sk[:])
            bcs_sb = gw.tile([P, BL], BF16, tag="bcssb")
            nc.scalar.copy(bcs_sb[:], bcs[:])
            for j in range(TPK):
                kt = k * TPK + j
                nc.vector.tensor_mul(wf_all[:, kt, :], es_list[j],
                                     bcs_sb[:])
    f_kn()
    f_inv()
    f_lhs2()

    # ============ Phase FE ===========================================
    with tc.tile_pool(name="evp", bufs=2) as evp, \
         tc.tile_pool(name="pfe", bufs=3, space="PSUM") as pfe:
        for mc in range(NT_C):
            evtt = evp.tile([P, KN], BF16, tag="evt")
            nc.gpsimd.dma_start(evtt[:], ccevt_out[mc])
            ps = pfe.tile([P, BL], F32, tag="feps")
            for kt in range(NT_KN):
                nc.tensor.matmul(
                    ps[:], evtt[:, kt * P:(kt + 1) * P],
                    wf_all[:, kt, :],
                    start=(kt == 0), stop=(kt == NT_KN - 1))
            nc.scalar.activation(fr_all[:, mc, :], ps[:], AF.Relu)
    f_wf()

    # ============ Phase OUT ==========================================
    with tc.tile_pool(name="ow", bufs=1) as ow, \
         tc.tile_pool(name="pout", bufs=1, space="PSUM") as pout:
        po = pout.tile([K, BL], F32)
        for j in range(NT_KV):
            rhs = vr_all[:, j, :] if j < NT_C else \
                fr_all[:, j - NT_C, :]
            nc.tensor.matmul(po[:], wo_sb[:, j * K:(j + 1) * K], rhs,
                             start=(j == 0), stop=(j == NT_KV - 1))
        osb = ow.tile([K, BL], F32)
        nc.scalar.activation(osb[:], po[:], AF.Identity,
                             bias=bout_sb[:])
        nc.sync.dma_start(outT[:], osb[:])
    f_fr()
    f_vr()
    _f6()
    _f5()
    _f4()
    _f3()
    _f2()
    _f1()
    _f0b()
    _f0()

    tc_cm.__exit__(None, None, None)
    nc.compile()
    return nc


def _tile_rows(a):
    """[NT_I*P, n] -> [P, NT_I, n]: row (i*P + p) -> [p, i]."""
    n = a.shape[1]
    return np.ascontiguousarray(
        a.reshape(NT_I, P, n).transpose(1, 0, 2))


def _host_prep(inputs):
    bf = ml_dtypes.bfloat16
    x_last = np.asarray(inputs["x"])[:, -1, :]  # [B, CH] f32
    wekT = np.asarray(inputs["WEk"]).T  # [CH, C]
    wevT = np.asarray(inputs["WEv"]).T
    shared = {
        "wkvt": _tile_rows(
            np.concatenate([inputs["Wk"], inputs["Wv"]], axis=0).T
        ).astype(bf),
        "statt": _tile_rows(
            np.asarray(inputs["static"]).transpose(1, 0, 2).reshape(CH, KN)
        ).astype(bf),
        "bkv": np.ascontiguousarray(
            np.concatenate([inputs["bk"], inputs["bv"]]).reshape(NT_KV, P).T),
        "ident": np.eye(P, dtype=bf),
        "wout": np.ascontiguousarray(
            np.asarray(inputs["Wout"]).T.reshape(NT_KV, P, K)
            .transpose(1, 0, 2).reshape(P, NT_KV * K)).astype(bf),
        "bws": np.asarray(inputs["bw"], dtype=np.float32).reshape(1, 1),
        "boutt": np.asarray(inputs["bout"], dtype=np.float32).reshape(K, 1),
    }
    in_maps = []
    for r in range(NCORES):
        cslc = slice(r * P, (r + 1) * P)
        m = dict(shared)
        m["xTt"] = _tile_rows(
            np.ascontiguousarray(x_last[r * BL:(r + 1) * BL].T)).astype(bf)
        m["wekt"] = _tile_rows(
            np.ascontiguousarray(wekT[:, cslc])).astype(bf)
        m["wevt"] = _tile_rows(
            np.ascontiguousarray(wevT[:, cslc])).astype(bf)
        m["bekc"] = np.ascontiguousarray(
            np.asarray(inputs["bEk"], dtype=np.float32)[cslc].reshape(P, 1))
        m["bevc"] = np.ascontiguousarray(
            np.asarray(inputs["bEv"], dtype=np.float32)[cslc].reshape(P, 1))
        m["wwc"] = np.ascontiguousarray(
            np.asarray(inputs["Ww"])[0, cslc].reshape(P, 1)).astype(bf)
        in_maps.append(m)
    return in_maps


def kernel(**inputs):
    if "nc" not in _CACHE:
        _CACHE["nc"] = _build_nc()
    nc = _CACHE["nc"]
    in_maps = _host_prep(inputs)
    res = bass_utils.run_bass_kernel_spmd(
        nc, in_maps, core_ids=list(range(NCORES)), trace=False)
    out = np.concatenate(
        [res.results[r]["outT"].T for r in range(NCORES)], axis=0)
    return np.ascontiguousarray(out[:, :, None], dtype=np.float32)


# revision 25
# speedup vs baseline: 1.7172x; 1.0212x over previous
"""Trainium2 Bass kernel for nn_Colar_static (retrieval_knn).

Strategy: data-parallel over batch B across 8 cores PLUS tensor-parallel
split of the Ek/Ev prototype projections over the C=1024 channel dim
(each core computes a [128, K*N] slab = 1/8 of the work the baseline
replicated). Slabs are exchanged with on-chip collectives:
  - AllReduce  [2, KN] f32   : Ek column sum-of-squares + wEv gate row
  - AllGather  [128, KN] bf16: Ek slab (c-tile per rank)
  - AllGather  [128, KN] bf16: Ev^T slab (kn on partitions, c-slice free)
Collectives overlap the batch-local k/v projection phase.

DMA descriptor *triggers* cost ~607ns each, serialized on the issuing
engine (SP or Activation are the only HW-DGE triggers). So all bulk
traffic uses host-retiled [128, i, n] layouts so each logical load is
ONE multi-dim DMA, and triggers are spread across the sync and scalar
queues. KV accumulates each PSUM bank to completion (q-outer) to avoid
the bank-cycling micro-idle penalty.

SBUF singles are created in reverse order of death (LIFO pool stack).
"""

import sys

for _p in ("/opt/trn_rl_repo", "/opt/pypackages"):
    if _p not in sys.path:
        sys.path.append(_p)

import numpy as np
import ml_dtypes

import concourse.bass as bass
import concourse.mybir as mybir
import concourse.tile as tile
from concourse import bacc
from concourse import bass_utils

B, T, CH, C, N, K = 4096, 8, 2048, 1024, 512, 5
NCORES = 8
BL = B // NCORES            # 512 batch rows per core
KN = K * N                  # 2560 prototype columns
P = 128
NT_I = CH // P              # 16 contraction tiles (input channels)
NT_C = C // P               # 8 tiles over C
NT_KN = KN // P             # 20 tiles over K*N
NT_KV = 2 * C // P          # 16 tiles over [k|v] output channels
TPK = NT_KN // K            # 4 kn-tiles per prototype
NCH = KN // 512             # 5 column chunks for the slab projections
EPS = 1e-8

F32 = mybir.dt.float32
BF16 = mybir.dt.bfloat16
AF = mybir.ActivationFunctionType

_CACHE = {}


def _build_nc():
    nc = bacc.Bacc(None, target_bir_lowering=False, debug=False)

    # [P, NT_I, n] host-retiled layouts: row (i*P + p) -> [p, i]
    xTt = nc.dram_tensor("xTt", [P, NT_I, BL], BF16, kind="ExternalInput")
    wkvt = nc.dram_tensor("wkvt", [P, NT_I, 2 * C], BF16,
                          kind="ExternalInput")
    wekt = nc.dram_tensor("wekt", [P, NT_I, P], BF16, kind="ExternalInput")
    wevt = nc.dram_tensor("wevt", [P, NT_I, P], BF16, kind="ExternalInput")
    statt = nc.dram_tensor("statt", [P, NT_I, KN], BF16,
                           kind="ExternalInput")
    bekc = nc.dram_tensor("bekc", [P, 1], F32, kind="ExternalInput")
    bevc = nc.dram_tensor("bevc", [P, 1], F32, kind="ExternalInput")
    wwc = nc.dram_tensor("wwc", [P, 1], BF16, kind="ExternalInput")
    bkv = nc.dram_tensor("bkv", [P, NT_KV], F32, kind="ExternalInput")
    ident = nc.dram_tensor("ident", [P, P], BF16, kind="ExternalInput")
    wout = nc.dram_tensor("wout", [P, NT_KV * K], BF16, kind="ExternalInput")
    bws = nc.dram_tensor("bws", [1, 1], F32, kind="ExternalInput")
    boutt = nc.dram_tensor("boutt", [K, 1], F32, kind="ExternalInput")
    outT = nc.dram_tensor("outT", [K, BL], F32, kind="ExternalOutput")

    # collective bounce buffers (inputs must be Internal-Local; gather
    # outputs Shared so ranks deposit slices into one HBM buffer)
    ccr_in = nc.dram_tensor("ccr_in", [2, KN], F32)
    ccr_out = nc.dram_tensor("ccr_out", [2, KN], F32)
    ccek_in = nc.dram_tensor("ccek_in", [P, KN], BF16)
    ccek_out = nc.dram_tensor("ccek_out", [NCORES, P, KN], BF16,
                              addr_space="Shared")
    ccevt_in = nc.dram_tensor("ccevt_in", [P, KN], BF16)
    ccevt_out = nc.dram_tensor("ccevt_out", [NCORES, P, KN], BF16,
                               addr_space="Shared")
    GROUPS = [list(range(NCORES))]

    tc_cm = tile.TileContext(nc)
    tc = tc_cm.__enter__()

    # ---- engine warmups: first use of an ACT table stalls; issue tiny
    # activations up front so table loads overlap the initial DMAs.
    warm, f_warm = tc.tile([1, 16], F32, name="warm")
    nc.vector.memset(warm[:], 1.0)
    for wf_i, wfunc in enumerate((AF.Identity, AF.Square, AF.Relu, AF.Exp,
                                  AF.Sqrt, AF.Sigmoid)):
        wo_t, f_wo_t = tc.tile([1, 16], F32, name=f"warmo{wf_i}")
        nc.scalar.activation(wo_t[:], warm[:], wfunc)
        f_wo_t()
    f_warm()

    # ---- persistents (die at the very end), bottom of pool stack
    epsb, _f0 = tc.tile([1, 1], F32, name="epsb")
    nc.vector.memset(epsb[:], EPS * EPS)
    epsb_p, _f0b = tc.tile([P, 1], F32, name="epsb_p")
    nc.vector.memset(epsb_p[:], EPS * EPS)
    ones_col, _f1 = tc.tile([P, 1], BF16, name="ones_col")
    nc.any.memset(ones_col[:], 1.0)
    ones_row, _f2 = tc.tile([1, P], F32, name="ones_row")
    nc.any.memset(ones_row[:], 1.0)
    bkv_sb, _f3 = tc.tile([P, NT_KV], F32, name="bkv_sb")
    nc.sync.dma_start(bkv_sb[:], bkv[:])
    bw_sb, _f4 = tc.tile([1, 1], F32, name="bw_sb")
    nc.sync.dma_start(bw_sb[:], bws[:])
    bout_sb, _f5 = tc.tile([K, 1], F32, name="bout_sb")
    nc.sync.dma_start(bout_sb[:], boutt[:])
    wo_sb, _f6 = tc.tile([P, NT_KV * K], BF16, name="wo_sb")
    nc.sync.dma_start(wo_sb[:], wout[:])

    # dies OUT-end
    vr_all, f_vr = tc.tile([P, NT_C, BL], BF16, name="vr_all")
    fr_all, f_fr = tc.tile([P, NT_C, BL], BF16, name="fr_all")
    # dies FE-end
    wf_all, f_wf = tc.tile([P, NT_KN, BL], BF16, name="wf_all")
    # die SIM-end (written after collectives)
    lhs2, f_lhs2 = tc.tile([P, NT_KN, 2], BF16, name="lhs2")
    inv_col, f_inv = tc.tile([P, NT_KN], F32, name="inv_col")
    kn_all, f_kn = tc.tile([P, NT_C, BL], BF16, name="kn_all")
    # die KV-end
    kT_all, f_kT = tc.tile([P, NT_C, BL], BF16, name="kT_all")
    sqk_all, f_sqk = tc.tile([P, NT_C, BL], BF16, name="sqk_all")
    xp_all, f_xp = tc.tile([P, NT_I, BL], BF16, name="xp_all")
    nc.gpsimd.dma_start(xp_all[:], xTt[:])
    # preloaded mid-P (see chunk loop) so the statf stream goes first
    wblk0, f_wblk0 = tc.tile([P, NT_I, 512], BF16, name="wblk0")
    wblk1, f_wblk1 = tc.tile([P, NT_I, 512], BF16, name="wblk1")

    # ============ Phase P: Ek / Ev^T slabs (this core's 128 c's) =====
    wek_sb, f_wek = tc.tile([P, NT_I, P], BF16, name="wek_sb")
    nc.sync.dma_start(wek_sb[:], wekt[:])
    wev_sb, f_wev = tc.tile([P, NT_I, P], BF16, name="wev_sb")
    nc.scalar.dma_start(wev_sb[:], wevt[:])
    bekc_sb, f_bek = tc.tile([P, 1], F32, name="bekc_sb")
    nc.scalar.dma_start(bekc_sb[:], bekc[:])
    bevc_sb, f_bev = tc.tile([P, 1], F32, name="bevc_sb")
    nc.scalar.dma_start(bevc_sb[:], bevc[:])
    wwc_sb, f_ww = tc.tile([P, 1], BF16, name="wwc_sb")
    nc.scalar.dma_start(wwc_sb[:], wwc[:])
    id_sb, f_id = tc.tile([P, P], BF16, name="id_sb")
    nc.scalar.dma_start(id_sb[:], ident[:])
    ek_slab, f_eks = tc.tile([P, KN], BF16, name="ek_slab")
    evt_slab, f_evs = tc.tile([P, KN], BF16, name="evt_slab")

    with tc.tile_pool(name="stp", bufs=2) as stp, \
         tc.tile_pool(name="pw", bufs=3) as pw, \
         tc.tile_pool(name="rowp", bufs=2) as rowp, \
         tc.tile_pool(name="ppk", bufs=2, space="PSUM") as ppk, \
         tc.tile_pool(name="prow", bufs=1, space="PSUM") as prow, \
         tc.tile_pool(name="ptp", bufs=2, space="PSUM") as ptp:
        for ch in range(NCH):
            cs = slice(ch * 512, (ch + 1) * 512)
            stch = stp.tile([P, NT_I, 512], BF16, tag="st")
            nc.sync.dma_start(stch[:, 0:NT_I // 2, :],
                              statt[:, 0:NT_I // 2, cs])
            nc.scalar.dma_start(stch[:, NT_I // 2:, :],
                                statt[:, NT_I // 2:, cs])
            if ch == NCH - 1:
                nc.sync.dma_start(wblk0[:], wkvt[:, :, 0:512])
                nc.scalar.dma_start(wblk1[:], wkvt[:, :, 512:1024])
            # Ek chunk
            ek_ps = ppk.tile([P, 512], F32, tag="ek")
            for i in range(NT_I):
                nc.tensor.matmul(ek_ps[:], wek_sb[:, i, :], stch[:, i, :],
                                 start=(i == 0), stop=(i == NT_I - 1))
            nc.scalar.activation(ek_slab[:, cs], ek_ps[:], AF.Identity,
                                 bias=bekc_sb[:])
            sqt = pw.tile([P, 512], BF16, tag="sq")
            nc.scalar.activation(sqt[:], ek_ps[:], AF.Square,
                                 bias=bekc_sb[:])
            sq_ps = prow.tile([1, 512], F32, tag="row")
            nc.tensor.matmul(sq_ps[:], ones_col[:], sqt[:])
            sq_row = rowp.tile([1, 512], F32, tag="sqr")
            nc.vector.tensor_copy(sq_row[:], sq_ps[:])
            nc.scalar.dma_start(ccr_in[0:1, cs], sq_row[:])
            # Ev chunk
            ev_ps = ppk.tile([P, 512], F32, tag="ev")
            for i in range(NT_I):
                nc.tensor.matmul(ev_ps[:], wev_sb[:, i, :], stch[:, i, :],
                                 start=(i == 0), stop=(i == NT_I - 1))
            evbf = pw.tile([P, 512], BF16, tag="ev")
            nc.scalar.activation(evbf[:], ev_ps[:], AF.Identity,
                                 bias=bevc_sb[:])
            wev_ps = prow.tile([1, 512], F32, tag="row")
            nc.tensor.matmul(wev_ps[:], wwc_sb[:], evbf[:])
            wev_row = rowp.tile([1, 512], F32, tag="wvr")
            nc.vector.tensor_copy(wev_row[:], wev_ps[:])
            nc.scalar.dma_start(ccr_in[1:2, cs], wev_row[:])
            # Ev^T chunk (4 PE transposes via identity)
            tp_ps = ptp.tile([P, 512], BF16, tag="tp")
            for q in range(4):
                nc.tensor.transpose(tp_ps[:, q * P:(q + 1) * P],
                                    evbf[:, q * P:(q + 1) * P], id_sb[:])
            nc.scalar.copy(evt_slab[:, cs], tp_ps[:])
        nc.sync.dma_start(ccek_in[:], ek_slab[:])
        nc.scalar.dma_start(ccevt_in[:], evt_slab[:])
    f_evs()
    f_eks()
    f_id()
    f_ww()
    f_bev()
    f_bek()
    f_wev()
    f_wek()

    # ============ Collectives (overlap the KV phase) ==================
    nc.gpsimd.collective_compute(
        "AllReduce", mybir.AluOpType.add, replica_groups=GROUPS,
        ins=[ccr_in[:].opt()], outs=[ccr_out[:].opt()])
    nc.gpsimd.collective_compute(
        "AllGather", mybir.AluOpType.bypass, replica_groups=GROUPS,
        ins=[ccek_in[:].opt()], outs=[ccek_out[:].opt()])
    nc.gpsimd.collective_compute(
        "AllGather", mybir.AluOpType.bypass, replica_groups=GROUPS,
        ins=[ccevt_in[:].opt()], outs=[ccevt_out[:].opt()])

    # inv_col = 1/sqrt(sum_sq + eps^2); wEv gate row -> lhs2[:, :, 1]
    with tc.tile_pool(name="colw", bufs=1) as colw:
        sq_col = colw.tile([P, NT_KN], F32, tag="sqc")
        nc.gpsimd.dma_start(
            sq_col[:], ccr_out[0, :].rearrange("(j p) -> p j", p=P))
        nrm = colw.tile([P, NT_KN], F32, tag="nrm")
        nc.scalar.activation(nrm[:], sq_col[:], AF.Sqrt, bias=epsb_p[:])
        nc.vector.reciprocal(inv_col[:], nrm[:])
        wev_col = colw.tile([P, NT_KN], F32, tag="wvc")
        nc.gpsimd.dma_start(
            wev_col[:], ccr_out[1, :].rearrange("(j p) -> p j", p=P))
        nc.any.memset(lhs2[:], 1.0)
        nc.vector.tensor_copy(lhs2[:, :, 1], wev_col[:])

    # ============ Phase KV: normalized kT, relu(vT) ==================
    # q-outer so each PSUM bank accumulates its 16 steps back-to-back.
    with tc.tile_pool(name="wkvp", bufs=2) as wkvp, \
         tc.tile_pool(name="pkv", bufs=4, space="PSUM") as pkv:
        for mg in range(4):
            if mg == 0:
                wblk = wblk0
            elif mg == 1:
                wblk = wblk1
            else:
                ms = slice(mg * 512, (mg + 1) * 512)
                wblk = wkvp.tile([P, NT_I, 512], BF16, tag="wb")
                nc.sync.dma_start(wblk[:, 0:NT_I // 2, :],
                                  wkvt[:, 0:NT_I // 2, ms])
                nc.scalar.dma_start(wblk[:, NT_I // 2:, :],
                                    wkvt[:, NT_I // 2:, ms])
            for q in range(4):
                m = mg * 4 + q
                kv_ps = pkv.tile([P, BL], F32, tag="kv", name=f"kv{m}")
                for i in range(NT_I):
                    nc.tensor.matmul(
                        kv_ps[:], wblk[:, i, q * P:(q + 1) * P],
                        xp_all[:, i, :],
                        start=(i == 0), stop=(i == NT_I - 1))
                if m < NT_C:
                    nc.scalar.activation(
                        kT_all[:, m, :], kv_ps[:], AF.Identity,
                        bias=bkv_sb[:, m:m + 1])
                    nc.scalar.activation(
                        sqk_all[:, m, :], kv_ps[:], AF.Square,
                        bias=bkv_sb[:, m:m + 1])
                else:
                    nc.scalar.activation(
                        vr_all[:, m - NT_C, :], kv_ps[:], AF.Relu,
                        bias=bkv_sb[:, m:m + 1])

    with tc.tile_pool(name="kvw", bufs=2) as kvw, \
         tc.tile_pool(name="pssk", bufs=1, space="PSUM") as pssk, \
         tc.tile_pool(name="pbc", bufs=1, space="PSUM") as pbc:
        ssk = pssk.tile([1, BL], F32)
        for m in range(NT_C):
            nc.tensor.matmul(ssk[:], ones_col[:], sqk_all[:, m, :],
                             start=(m == 0), stop=(m == NT_C - 1))
        nk = kvw.tile([1, BL], F32, tag="nk")
        nc.scalar.activation(nk[:], ssk[:], AF.Sqrt, bias=epsb[:])
        invk = kvw.tile([1, BL], F32, tag="invk")
        nc.vector.reciprocal(invk[:], nk[:])
        bc = pbc.tile([P, BL], F32)
        nc.tensor.matmul(bc[:], ones_row[:], invk[:])
        for m in range(NT_C):
            nc.vector.tensor_mul(kn_all[:, m, :], kT_all[:, m, :],
                                 bc[:])
    f_wblk1()
    f_wblk0()
    f_xp()
    f_sqk()
    f_kT()

    # ============ Fused SIM + GATE + WF ==============================
    with tc.tile_pool(name="ekp", bufs=2) as ekp, \
         tc.tile_pool(name="gw", bufs=2) as gw, \
         tc.tile_pool(name="esw", bufs=8) as esw, \
         tc.tile_pool(name="psim", bufs=3, space="PSUM") as psim, \
         tc.tile_pool(name="pg", bufs=2, space="PSUM") as pg, \
         tc.tile_pool(name="pbc2", bufs=1, space="PSUM") as pbc2:
        for k in range(K):
            ks = slice(k * 512, (k + 1) * 512)
            ekt = ekp.tile([P, NT_C, 512], BF16, tag="ek")
            nc.gpsimd.dma_start(
                ekt[:], ccek_out[:, :, ks].rearrange("m p c -> p m c"))
            gse = pg.tile([1, BL], F32, tag="gse")
            gtg = pg.tile([1, BL], F32, tag="gtg")
            es_list = []
            for j in range(TPK):
                kt = k * TPK + j
                ps = psim.tile([P, BL], F32, tag="simps")
                for m in range(NT_C):
                    nc.tensor.matmul(
                        ps[:], ekt[:, m, j * P:(j + 1) * P],
                        kn_all[:, m, :],
                        start=(m == 0), stop=(m == NT_C - 1))
                es = esw.tile([P, BL], BF16, tag="esw")
                nc.scalar.activation(es[:], ps[:], AF.Exp,
                                     scale=inv_col[:, kt:kt + 1])
                es_list.append(es)
                nc.tensor.matmul(gse[:], lhs2[:, kt, 0:1], es[:],
                                 start=(j == 0), stop=(j == TPK - 1))
                nc.tensor.matmul(gtg[:], lhs2[:, kt, 1:2], es[:],
                                 start=(j == 0), stop=(j == TPK - 1))
            rs = gw.tile([1, BL], F32, tag="rs")
            nc.vector.reciprocal(rs[:], gse[:])
            tg = gw.tile([1, BL], F32, tag="tg")
            nc.vector.tensor_mul(tg[:], gtg[:], rs[:])
            fwk = gw.tile([1, BL], F32, tag="fwk")
            nc.scalar.activation(fwk[:], tg[:], AF.Sigmoid,
                                 bias=bw_sb[0:1, 0:1])
            sk = gw.tile([1, BL], F32, tag="sk")
            nc.vector.tensor_mul(sk[:], fwk[:], rs[:])
            bcs = pbc2.tile([P, BL], F32, tag="bcs")
            nc.tensor.matmul(bcs[:], ones_row[:], sk[:])
            bcs_sb = gw.tile([P, BL], BF16, tag="bcssb")
            nc.scalar.copy(bcs_sb[:], bcs[:])
            for j in range(TPK):
                kt = k * TPK + j
                nc.vector.tensor_mul(wf_all[:, kt, :], es_list[j],
                                     bcs_sb[:])
    f_kn()
    f_inv()
    f_lhs2()

    # ============ Phase FE ===========================================
    with tc.tile_pool(name="evp", bufs=2) as evp, \
         tc.tile_pool(name="pfe", bufs=3, space="PSUM") as pfe:
        for mc in range(NT_C):
            evtt = evp.tile([P, KN], BF16, tag="evt")
            nc.gpsimd.dma_start(evtt[:], ccevt_out[mc])
            ps = pfe.tile([P, BL], F32, tag="feps")
            for kt in range(NT_KN):
                nc.tensor.matmul(
                    ps[:], evtt[:, kt * P:(kt + 1) * P],
                    wf_all[:, kt, :],
                    start=(kt == 0), stop=(kt == NT_KN - 1))
            nc.scalar.activation(fr_all[:, mc, :], ps[:], AF.Relu)
    f_wf()

    # ============ Phase OUT ==========================================
    with tc.tile_pool(name="ow", bufs=1) as ow, \
         tc.tile_pool(name="pout", bufs=1, space="PSUM") as pout:
        po = pout.tile([K, BL], F32)
        for j in range(NT_KV):
            rhs = vr_all[:, j, :] if j < NT_C else \
                fr_all[:, j - NT_C, :]
            nc.tensor.matmul(po[:], wo_sb[:, j * K:(j + 1) * K], rhs,
                             start=(j == 0), stop=(j == NT_KV - 1))
        osb = ow.tile([K, BL], F32)
        nc.scalar.activation(osb[:], po[:], AF.Identity,
                             bias=bout_sb[:])
        nc.sync.dma_start(outT[:], osb[:])
    f_fr()
    f_vr()
    _f6()
    _f5()
    _f4()
    _f3()
    _f2()
    _f1()
    _f0b()
    _f0()

    tc_cm.__exit__(None, None, None)
    nc.compile()
    return nc


def _tile_rows(a):
    """[NT_I*P, n] -> [P, NT_I, n]: row (i*P + p) -> [p, i]."""
    n = a.shape[1]
    return np.ascontiguousarray(
        a.reshape(NT_I, P, n).transpose(1, 0, 2))


def _host_prep(inputs):
    bf = ml_dtypes.bfloat16
    x_last = np.asarray(inputs["x"])[:, -1, :]  # [B, CH] f32
    wekT = np.asarray(inputs["WEk"]).T  # [CH, C]
    wevT = np.asarray(inputs["WEv"]).T
    shared = {
        "wkvt": _tile_rows(
            np.concatenate([inputs["Wk"], inputs["Wv"]], axis=0).T
        ).astype(bf),
        "statt": _tile_rows(
            np.asarray(inputs["static"]).transpose(1, 0, 2).reshape(CH, KN)
        ).astype(bf),
        "bkv": np.ascontiguousarray(
            np.concatenate([inputs["bk"], inputs["bv"]]).reshape(NT_KV, P).T),
        "ident": np.eye(P, dtype=bf),
        "wout": np.ascontiguousarray(
            np.asarray(inputs["Wout"]).T.reshape(NT_KV, P, K)
            .transpose(1, 0, 2).reshape(P, NT_KV * K)).astype(bf),
        "bws": np.asarray(inputs["bw"], dtype=np.float32).reshape(1, 1),
        "boutt": np.asarray(inputs["bout"], dtype=np.float32).reshape(K, 1),
    }
    in_maps = []
    for r in range(NCORES):
        cslc = slice(r * P, (r + 1) * P)
        m = dict(shared)
        m["xTt"] = _tile_rows(
            np.ascontiguousarray(x_last[r * BL:(r + 1) * BL].T)).astype(bf)
        m["wekt"] = _tile_rows(
            np.ascontiguousarray(wekT[:, cslc])).astype(bf)
        m["wevt"] = _tile_rows(
            np.ascontiguousarray(wevT[:, cslc])).astype(bf)
        m["bekc"] = np.ascontiguousarray(
            np.asarray(inputs["bEk"], dtype=np.float32)[cslc].reshape(P, 1))
        m["bevc"] = np.ascontiguousarray(
            np.asarray(inputs["bEv"], dtype=np.float32)[cslc].reshape(P, 1))
        m["wwc"] = np.ascontiguousarray(
            np.asarray(inputs["Ww"])[0, cslc].reshape(P, 1)).astype(bf)
        in_maps.append(m)
    return in_maps


def kernel(**inputs):
    if "nc" not in _CACHE:
        _CACHE["nc"] = _build_nc()
    nc = _CACHE["nc"]
    in_maps = _host_prep(inputs)
    res = bass_utils.run_bass_kernel_spmd(
        nc, in_maps, core_ids=list(range(NCORES)), trace=False)
    out = np.concatenate(
        [res.results[r]["outT"].T for r in range(NCORES)], axis=0)
    return np.ascontiguousarray(out[:, :, None], dtype=np.float32)


# revision 28
# speedup vs baseline: 1.9900x; 1.1589x over previous
"""Trainium2 Bass kernel for nn_Colar_static (retrieval_knn).

Strategy: data-parallel over batch B across 8 cores PLUS tensor-parallel
split of the Ek/Ev prototype projections over the C=1024 channel dim
(each core computes a [128, K*N] slab = 1/8 of the work the baseline
replicated). Slabs are exchanged with on-chip collectives:
  - AllReduce  [2, KN] f32   : Ek column sum-of-squares + wEv gate row
  - AllGather  [128, KN] bf16: Ek slab (c-tile per rank)
  - AllGather  [128, KN] bf16: Ev^T slab (kn on partitions, c-slice free)
Collectives overlap the batch-local k/v projection phase.

DMA descriptor *triggers* cost ~607ns each, serialized on the issuing
engine (SP or Activation are the only HW-DGE triggers). So all bulk
traffic uses host-retiled [128, i, n] layouts so each logical load is
ONE multi-dim DMA, and triggers are spread across the sync and scalar
queues. KV accumulates each PSUM bank to completion (q-outer) to avoid
the bank-cycling micro-idle penalty.

SBUF singles are created in reverse order of death (LIFO pool stack).
"""

import sys

for _p in ("/opt/trn_rl_repo", "/opt/pypackages"):
    if _p not in sys.path:
        sys.path.append(_p)

import numpy as np
import ml_dtypes

import concourse.bass as bass
import concourse.mybir as mybir
import concourse.tile as tile
from concourse import bacc
from concourse import bass_utils

B, T, CH, C, N, K = 4096, 8, 2048, 1024, 512, 5
NCORES = 8
BL = B // NCORES            # 512 batch rows per core
KN = K * N                  # 2560 prototype columns
P = 128
NT_I = CH // P              # 16 contraction tiles (input channels)
NT_C = C // P               # 8 tiles over C
NT_KN = KN // P             # 20 tiles over K*N
NT_KV = 2 * C // P          # 16 tiles over [k|v] output channels
TPK = NT_KN // K            # 4 kn-tiles per prototype
NCH = KN // 512             # 5 column chunks for the slab projections
EPS = 1e-8

F32 = mybir.dt.float32
BF16 = mybir.dt.bfloat16
AF = mybir.ActivationFunctionType

_CACHE = {}


def _build_nc():
    nc = bacc.Bacc(None, target_bir_lowering=False, debug=False)

    # [P, NT_I, n] host-retiled layouts: row (i*P + p) -> [p, i]
    xTt = nc.dram_tensor("xTt", [P, NT_I, BL], BF16, kind="ExternalInput")
    wkvt = nc.dram_tensor("wkvt", [P, NT_I, 2 * C], BF16,
                          kind="ExternalInput")
    wekt = nc.dram_tensor("wekt", [P, NT_I, P], BF16, kind="ExternalInput")
    wevt = nc.dram_tensor("wevt", [P, NT_I, P], BF16, kind="ExternalInput")
    statt = nc.dram_tensor("statt", [P, NT_I, KN], BF16,
                           kind="ExternalInput")
    bekc = nc.dram_tensor("bekc", [P, 1], F32, kind="ExternalInput")
    bevc = nc.dram_tensor("bevc", [P, 1], F32, kind="ExternalInput")
    wwc = nc.dram_tensor("wwc", [P, 1], BF16, kind="ExternalInput")
    bkv = nc.dram_tensor("bkv", [P, NT_KV], F32, kind="ExternalInput")
    ident = nc.dram_tensor("ident", [P, P], BF16, kind="ExternalInput")
    wout = nc.dram_tensor("wout", [P, NT_KV * K], BF16, kind="ExternalInput")
    bws = nc.dram_tensor("bws", [1, 1], F32, kind="ExternalInput")
    boutt = nc.dram_tensor("boutt", [K, 1], F32, kind="ExternalInput")
    outT = nc.dram_tensor("outT", [K, BL], F32, kind="ExternalOutput")

    # collective bounce buffers (inputs must be Internal-Local; gather
    # outputs Shared so ranks deposit slices into one HBM buffer)
    ccr_in = nc.dram_tensor("ccr_in", [2, KN], F32)
    ccr_out = nc.dram_tensor("ccr_out", [2, KN], F32)
    ccek_in = nc.dram_tensor("ccek_in", [P, KN], BF16)
    ccek_out = nc.dram_tensor("ccek_out", [NCORES, P, KN], BF16,
                              addr_space="Shared")
    ccevt_in = nc.dram_tensor("ccevt_in", [P, KN], BF16)
    ccevt_out = nc.dram_tensor("ccevt_out", [NCORES, P, KN], BF16,
                               addr_space="Shared")
    GROUPS = [list(range(NCORES))]

    tc_cm = tile.TileContext(nc)
    tc = tc_cm.__enter__()

    # ---- engine warmups: first use of an ACT table stalls; issue tiny
    # activations up front so table loads overlap the initial DMAs.
    warm, f_warm = tc.tile([1, 16], F32, name="warm")
    nc.vector.memset(warm[:], 1.0)
    for wf_i, wfunc in enumerate((AF.Identity, AF.Square, AF.Relu, AF.Exp,
                                  AF.Sqrt, AF.Sigmoid)):
        wo_t, f_wo_t = tc.tile([1, 16], F32, name=f"warmo{wf_i}")
        nc.scalar.activation(wo_t[:], warm[:], wfunc)
        f_wo_t()
    f_warm()

    # ---- persistents (die at the very end), bottom of pool stack
    epsb, _f0 = tc.tile([1, 1], F32, name="epsb")
    nc.vector.memset(epsb[:], EPS * EPS)
    epsb_p, _f0b = tc.tile([P, 1], F32, name="epsb_p")
    nc.vector.memset(epsb_p[:], EPS * EPS)
    ones_col, _f1 = tc.tile([P, 1], BF16, name="ones_col")
    nc.any.memset(ones_col[:], 1.0)
    ones_row, _f2 = tc.tile([1, P], F32, name="ones_row")
    nc.any.memset(ones_row[:], 1.0)
    bkv_sb, _f3 = tc.tile([P, NT_KV], F32, name="bkv_sb")
    nc.sync.dma_start(bkv_sb[:], bkv[:])
    bw_sb, _f4 = tc.tile([1, 1], F32, name="bw_sb")
    nc.sync.dma_start(bw_sb[:], bws[:])
    bout_sb, _f5 = tc.tile([K, 1], F32, name="bout_sb")
    nc.sync.dma_start(bout_sb[:], boutt[:])
    wo_sb, _f6 = tc.tile([P, NT_KV * K], BF16, name="wo_sb")
    nc.sync.dma_start(wo_sb[:], wout[:])

    # dies OUT-end
    vr_all, f_vr = tc.tile([P, NT_C, BL], BF16, name="vr_all")
    fr_all, f_fr = tc.tile([P, NT_C, BL], BF16, name="fr_all")
    # dies FE-end
    wf_all, f_wf = tc.tile([P, NT_KN, BL], BF16, name="wf_all")
    # die SIM-end (written after collectives)
    lhs2, f_lhs2 = tc.tile([P, NT_KN, 2], BF16, name="lhs2")
    inv_col, f_inv = tc.tile([P, NT_KN], F32, name="inv_col")
    kn_all, f_kn = tc.tile([P, NT_C, BL], BF16, name="kn_all")
    # die KV-end
    kT_all, f_kT = tc.tile([P, NT_C, BL], BF16, name="kT_all")
    sqk_all, f_sqk = tc.tile([P, NT_C, BL], BF16, name="sqk_all")
    xp_all, f_xp = tc.tile([P, NT_I, BL], BF16, name="xp_all")
    nc.gpsimd.dma_start(xp_all[:], xTt[:])
    # preloaded mid-P (see chunk loop) so the statf stream goes first
    wblk0, f_wblk0 = tc.tile([P, NT_I, 512], BF16, name="wblk0")
    wblk1, f_wblk1 = tc.tile([P, NT_I, 512], BF16, name="wblk1")

    # ============ Phase P: Ek / Ev^T slabs (this core's 128 c's) =====
    wek_sb, f_wek = tc.tile([P, NT_I, P], BF16, name="wek_sb")
    nc.sync.dma_start(wek_sb[:], wekt[:])
    wev_sb, f_wev = tc.tile([P, NT_I, P], BF16, name="wev_sb")
    nc.scalar.dma_start(wev_sb[:], wevt[:])
    bekc_sb, f_bek = tc.tile([P, 1], F32, name="bekc_sb")
    nc.scalar.dma_start(bekc_sb[:], bekc[:])
    bevc_sb, f_bev = tc.tile([P, 1], F32, name="bevc_sb")
    nc.scalar.dma_start(bevc_sb[:], bevc[:])
    wwc_sb, f_ww = tc.tile([P, 1], BF16, name="wwc_sb")
    nc.scalar.dma_start(wwc_sb[:], wwc[:])
    id_sb, f_id = tc.tile([P, P], BF16, name="id_sb")
    nc.scalar.dma_start(id_sb[:], ident[:])
    ek_slab, f_eks = tc.tile([P, KN], BF16, name="ek_slab")
    evt_slab, f_evs = tc.tile([P, KN], BF16, name="evt_slab")

    with tc.tile_pool(name="stp", bufs=2) as stp, \
         tc.tile_pool(name="pw", bufs=3) as pw, \
         tc.tile_pool(name="rowp", bufs=2) as rowp, \
         tc.tile_pool(name="ppk", bufs=2, space="PSUM") as ppk, \
         tc.tile_pool(name="prow", bufs=1, space="PSUM") as prow, \
         tc.tile_pool(name="ptp", bufs=2, space="PSUM") as ptp:
        for ch in range(NCH):
            cs = slice(ch * 512, (ch + 1) * 512)
            stch = stp.tile([P, NT_I, 512], BF16, tag="st")
            nc.sync.dma_start(stch[:, 0:NT_I // 2, :],
                              statt[:, 0:NT_I // 2, cs])
            nc.scalar.dma_start(stch[:, NT_I // 2:, :],
                                statt[:, NT_I // 2:, cs])
            if ch == NCH - 1:
                nc.sync.dma_start(wblk0[:], wkvt[:, :, 0:512])
                nc.scalar.dma_start(wblk1[:], wkvt[:, :, 512:1024])
            # Ek chunk
            ek_ps = ppk.tile([P, 512], F32, tag="ek")
            for i in range(NT_I):
                nc.tensor.matmul(ek_ps[:], wek_sb[:, i, :], stch[:, i, :],
                                 start=(i == 0), stop=(i == NT_I - 1))
            nc.scalar.activation(ek_slab[:, cs], ek_ps[:], AF.Identity,
                                 bias=bekc_sb[:])
            sqt = pw.tile([P, 512], BF16, tag="sq")
            nc.scalar.activation(sqt[:], ek_ps[:], AF.Square,
                                 bias=bekc_sb[:])
            sq_ps = prow.tile([1, 512], F32, tag="row")
            nc.tensor.matmul(sq_ps[:], ones_col[:], sqt[:])
            sq_row = rowp.tile([1, 512], F32, tag="sqr")
            nc.vector.tensor_copy(sq_row[:], sq_ps[:])
            nc.scalar.dma_start(ccr_in[0:1, cs], sq_row[:])
            # Ev chunk
            ev_ps = ppk.tile([P, 512], F32, tag="ev")
            for i in range(NT_I):
                nc.tensor.matmul(ev_ps[:], wev_sb[:, i, :], stch[:, i, :],
                                 start=(i == 0), stop=(i == NT_I - 1))
            evbf = pw.tile([P, 512], BF16, tag="ev")
            nc.scalar.activation(evbf[:], ev_ps[:], AF.Identity,
                                 bias=bevc_sb[:])
            wev_ps = prow.tile([1, 512], F32, tag="row")
            nc.tensor.matmul(wev_ps[:], wwc_sb[:], evbf[:])
            wev_row = rowp.tile([1, 512], F32, tag="wvr")
            nc.vector.tensor_copy(wev_row[:], wev_ps[:])
            nc.scalar.dma_start(ccr_in[1:2, cs], wev_row[:])
            # Ev^T chunk (4 PE transposes via identity)
            tp_ps = ptp.tile([P, 512], BF16, tag="tp")
            for q in range(4):
                nc.tensor.transpose(tp_ps[:, q * P:(q + 1) * P],
                                    evbf[:, q * P:(q + 1) * P], id_sb[:])
            nc.scalar.copy(evt_slab[:, cs], tp_ps[:])
        nc.sync.dma_start(ccek_in[:], ek_slab[:])
        nc.scalar.dma_start(ccevt_in[:], evt_slab[:])
    f_evs()
    f_eks()
    f_id()
    f_ww()
    f_bev()
    f_bek()
    f_wev()
    f_wek()

    # KV weight blocks 2/3: issue the loads BEFORE the collectives —
    # HW-DGE triggers queued after a collective serialize behind it.
    kvw_cm = tc.tile_pool(name="wkvp", bufs=2)
    wkvp = kvw_cm.__enter__()
    wblk23 = []
    for mg in (2, 3):
        ms = slice(mg * 512, (mg + 1) * 512)
        wb = wkvp.tile([P, NT_I, 512], BF16, tag="wb", name=f"wb{mg}")
        nc.sync.dma_start(wb[:, 0:NT_I // 2, :], wkvt[:, 0:NT_I // 2, ms])
        nc.scalar.dma_start(wb[:, NT_I // 2:, :], wkvt[:, NT_I // 2:, ms])
        wblk23.append(wb)

    # ============ Collectives (overlap the KV phase) ==================
    nc.gpsimd.collective_compute(
        "AllReduce", mybir.AluOpType.add, replica_groups=GROUPS,
        ins=[ccr_in[:].opt()], outs=[ccr_out[:].opt()])
    nc.gpsimd.collective_compute(
        "AllGather", mybir.AluOpType.bypass, replica_groups=GROUPS,
        ins=[ccek_in[:].opt()], outs=[ccek_out[:].opt()])
    nc.gpsimd.collective_compute(
        "AllGather", mybir.AluOpType.bypass, replica_groups=GROUPS,
        ins=[ccevt_in[:].opt()], outs=[ccevt_out[:].opt()])

    # inv_col = 1/sqrt(sum_sq + eps^2); wEv gate row -> lhs2[:, :, 1]
    with tc.tile_pool(name="colw", bufs=1) as colw:
        sq_col = colw.tile([P, NT_KN], F32, tag="sqc")
        nc.gpsimd.dma_start(
            sq_col[:], ccr_out[0, :].rearrange("(j p) -> p j", p=P))
        nrm = colw.tile([P, NT_KN], F32, tag="nrm")
        nc.scalar.activation(nrm[:], sq_col[:], AF.Sqrt, bias=epsb_p[:])
        nc.vector.reciprocal(inv_col[:], nrm[:])
        wev_col = colw.tile([P, NT_KN], F32, tag="wvc")
        nc.gpsimd.dma_start(
            wev_col[:], ccr_out[1, :].rearrange("(j p) -> p j", p=P))
        nc.any.memset(lhs2[:], 1.0)
        nc.vector.tensor_copy(lhs2[:, :, 1], wev_col[:])

    # ============ Phase KV: normalized kT, relu(vT) ==================
    # q-outer so each PSUM bank accumulates its 16 steps back-to-back;
    # the k sum-of-squares reduction is folded into the loop.
    with tc.tile_pool(name="pkv", bufs=3, space="PSUM") as pkv, \
         tc.tile_pool(name="pssk", bufs=1, space="PSUM") as pssk:
        ssk = pssk.tile([1, BL], F32)
        for mg in range(4):
            wblk = (wblk0, wblk1, wblk23[0], wblk23[1])[mg]
            for q in range(4):
                m = mg * 4 + q
                kv_ps = pkv.tile([P, BL], F32, tag="kv", name=f"kv{m}")
                for i in range(NT_I):
                    nc.tensor.matmul(
                        kv_ps[:], wblk[:, i, q * P:(q + 1) * P],
                        xp_all[:, i, :],
                        start=(i == 0), stop=(i == NT_I - 1))
                if m < NT_C:
                    nc.scalar.activation(
                        kT_all[:, m, :], kv_ps[:], AF.Identity,
                        bias=bkv_sb[:, m:m + 1])
                    nc.scalar.activation(
                        sqk_all[:, m, :], kv_ps[:], AF.Square,
                        bias=bkv_sb[:, m:m + 1])
                    nc.tensor.matmul(ssk[:], ones_col[:],
                                     sqk_all[:, m, :],
                                     start=(m == 0), stop=(m == NT_C - 1))
                else:
                    nc.scalar.activation(
                        vr_all[:, m - NT_C, :], kv_ps[:], AF.Relu,
                        bias=bkv_sb[:, m:m + 1])
        with tc.tile_pool(name="kvw", bufs=2) as kvw, \
             tc.tile_pool(name="pbc", bufs=1, space="PSUM") as pbc:
            nk = kvw.tile([1, BL], F32, tag="nk")
            nc.scalar.activation(nk[:], ssk[:], AF.Sqrt, bias=epsb[:])
            invk = kvw.tile([1, BL], F32, tag="invk")
            nc.vector.reciprocal(invk[:], nk[:])
            bc = pbc.tile([P, BL], F32)
            nc.tensor.matmul(bc[:], ones_row[:], invk[:])
            for m in range(NT_C):
                nc.vector.tensor_mul(kn_all[:, m, :], kT_all[:, m, :],
                                     bc[:])
    kvw_cm.__exit__(None, None, None)
    f_wblk1()
    f_wblk0()
    f_xp()
    f_sqk()
    f_kT()

    # ============ Fused SIM + GATE + WF ==============================
    with tc.tile_pool(name="ekp", bufs=2) as ekp, \
         tc.tile_pool(name="gw", bufs=2) as gw, \
         tc.tile_pool(name="esw", bufs=8) as esw, \
         tc.tile_pool(name="psim", bufs=3, space="PSUM") as psim, \
         tc.tile_pool(name="pg", bufs=2, space="PSUM") as pg, \
         tc.tile_pool(name="pbc2", bufs=1, space="PSUM") as pbc2:
        for k in range(K):
            ks = slice(k * 512, (k + 1) * 512)
            ekt = ekp.tile([P, NT_C, 512], BF16, tag="ek")
            nc.gpsimd.dma_start(
                ekt[:], ccek_out[:, :, ks].rearrange("m p c -> p m c"))
            gse = pg.tile([1, BL], F32, tag="gse")
            gtg = pg.tile([1, BL], F32, tag="gtg")
            es_list = []
            for j in range(TPK):
                kt = k * TPK + j
                ps = psim.tile([P, BL], F32, tag="simps")
                for m in range(NT_C):
                    nc.tensor.matmul(
                        ps[:], ekt[:, m, j * P:(j + 1) * P],
                        kn_all[:, m, :],
                        start=(m == 0), stop=(m == NT_C - 1))
                es = esw.tile([P, BL], BF16, tag="esw")
                nc.scalar.activation(es[:], ps[:], AF.Exp,
                                     scale=inv_col[:, kt:kt + 1])
                es_list.append(es)
                nc.tensor.matmul(gse[:], lhs2[:, kt, 0:1], es[:],
                                 start=(j == 0), stop=(j == TPK - 1))
                nc.tensor.matmul(gtg[:], lhs2[:, kt, 1:2], es[:],
                                 start=(j == 0), stop=(j == TPK - 1))
            rs = gw.tile([1, BL], F32, tag="rs")
            nc.vector.reciprocal(rs[:], gse[:])
            tg = gw.tile([1, BL], F32, tag="tg")
            nc.vector.tensor_mul(tg[:], gtg[:], rs[:])
            fwk = gw.tile([1, BL], F32, tag="fwk")
            nc.scalar.activation(fwk[:], tg[:], AF.Sigmoid,
                                 bias=bw_sb[0:1, 0:1])
            sk = gw.tile([1, BL], F32, tag="sk")
            nc.vector.tensor_mul(sk[:], fwk[:], rs[:])
            bcs = pbc2.tile([P, BL], F32, tag="bcs")
            nc.tensor.matmul(bcs[:], ones_row[:], sk[:])
            bcs_sb = gw.tile([P, BL], BF16, tag="bcssb")
            nc.scalar.copy(bcs_sb[:], bcs[:])
            for j in range(TPK):
                kt = k * TPK + j
                nc.vector.tensor_mul(wf_all[:, kt, :], es_list[j],
                                     bcs_sb[:])
    f_kn()
    f_inv()
    f_lhs2()

    # ============ Phase FE ===========================================
    with tc.tile_pool(name="evp", bufs=2) as evp, \
         tc.tile_pool(name="pfe", bufs=3, space="PSUM") as pfe:
        for mc in range(NT_C):
            evtt = evp.tile([P, KN], BF16, tag="evt")
            nc.gpsimd.dma_start(evtt[:], ccevt_out[mc])
            ps = pfe.tile([P, BL], F32, tag="feps")
            for kt in range(NT_KN):
                nc.tensor.matmul(
                    ps[:], evtt[:, kt * P:(kt + 1) * P],
                    wf_all[:, kt, :],
                    start=(kt == 0), stop=(kt == NT_KN - 1))
            nc.scalar.activation(fr_all[:, mc, :], ps[:], AF.Relu)
    f_wf()

    # ============ Phase OUT ==========================================
    with tc.tile_pool(name="ow", bufs=1) as ow, \
         tc.tile_pool(name="pout", bufs=1, space="PSUM") as pout:
        po = pout.tile([K, BL], F32)
        for j in range(NT_KV):
            rhs = vr_all[:, j, :] if j < NT_C else \
                fr_all[:, j - NT_C, :]
            nc.tensor.matmul(po[:], wo_sb[:, j * K:(j + 1) * K], rhs,
                             start=(j == 0), stop=(j == NT_KV - 1))
        osb = ow.tile([K, BL], F32)
        nc.scalar.activation(osb[:], po[:], AF.Identity,
                             bias=bout_sb[:])
        nc.sync.dma_start(outT[:], osb[:])
    f_fr()
    f_vr()
    _f6()
    _f5()
    _f4()
    _f3()
    _f2()
    _f1()
    _f0b()
    _f0()

    tc_cm.__exit__(None, None, None)
    nc.compile()
    return nc


def _tile_rows(a):
    """[NT_I*P, n] -> [P, NT_I, n]: row (i*P + p) -> [p, i]."""
    n = a.shape[1]
    return np.ascontiguousarray(
        a.reshape(NT_I, P, n).transpose(1, 0, 2))


def _host_prep(inputs):
    bf = ml_dtypes.bfloat16
    x_last = np.asarray(inputs["x"])[:, -1, :]  # [B, CH] f32
    wekT = np.asarray(inputs["WEk"]).T  # [CH, C]
    wevT = np.asarray(inputs["WEv"]).T
    shared = {
        "wkvt": _tile_rows(
            np.concatenate([inputs["Wk"], inputs["Wv"]], axis=0).T
        ).astype(bf),
        "statt": _tile_rows(
            np.asarray(inputs["static"]).transpose(1, 0, 2).reshape(CH, KN)
        ).astype(bf),
        "bkv": np.ascontiguousarray(
            np.concatenate([inputs["bk"], inputs["bv"]]).reshape(NT_KV, P).T),
        "ident": np.eye(P, dtype=bf),
        "wout": np.ascontiguousarray(
            np.asarray(inputs["Wout"]).T.reshape(NT_KV, P, K)
            .transpose(1, 0, 2).reshape(P, NT_KV * K)).astype(bf),
        "bws": np.asarray(inputs["bw"], dtype=np.float32).reshape(1, 1),
        "boutt": np.asarray(inputs["bout"], dtype=np.float32).reshape(K, 1),
    }
    in_maps = []
    for r in range(NCORES):
        cslc = slice(r * P, (r + 1) * P)
        m = dict(shared)
        m["xTt"] = _tile_rows(
            np.ascontiguousarray(x_last[r * BL:(r + 1) * BL].T)).astype(bf)
        m["wekt"] = _tile_rows(
            np.ascontiguousarray(wekT[:, cslc])).astype(bf)
        m["wevt"] = _tile_rows(
            np.ascontiguousarray(wevT[:, cslc])).astype(bf)
        m["bekc"] = np.ascontiguousarray(
            np.asarray(inputs["bEk"], dtype=np.float32)[cslc].reshape(P, 1))
        m["bevc"] = np.ascontiguousarray(
            np.asarray(inputs["bEv"], dtype=np.float32)[cslc].reshape(P, 1))
        m["wwc"] = np.ascontiguousarray(
            np.asarray(inputs["Ww"])[0, cslc].reshape(P, 1)).astype(bf)
        in_maps.append(m)
    return in_maps


def kernel(**inputs):
    if "nc" not in _CACHE:
        _CACHE["nc"] = _build_nc()
    nc = _CACHE["nc"]
    in_maps = _host_prep(inputs)
    res = bass_utils.run_bass_kernel_spmd(
        nc, in_maps, core_ids=list(range(NCORES)), trace=False)
    out = np.concatenate(
        [res.results[r]["outT"].T for r in range(NCORES)], axis=0)
    return np.ascontiguousarray(out[:, :, None], dtype=np.float32)
